# revision 1
# baseline (speedup 1.0000x reference)
"""GPT-2-ish forward (B=4, T=1024, D=768, H=12, L=2, V=50257) on 8 trn2 cores.

Sharding: core pair (2b, 2b+1) both run the full trunk for batch b
(replicated, zero collectives); lm_head is vocab-split within the pair
(each core does 25600 of the host-padded 51200 vocab columns).

On-device layout: activations transposed [features, tokens]. Attention
uses attT [keys, q] as the stationary matmul operand with a ones-column
appended to V so the softmax denominator lands in the free dim of the
(att @ V_aug) output; normalization is then a tensor_scalar_mul.
LayerNorm stats via ones-vector matmuls (contraction over partitions);
(g*rstd) / (b - g*mean*rstd) broadcasts built as rank-1 matmuls in PSUM.
All matmuls bf16 with fp32 PSUM accumulation; residual stream fp32 in
SBUF; logits evicted fp16 and upcast on host.
"""

import numpy as np
import ml_dtypes
from contextlib import ExitStack

import concourse.bass as bass
from concourse import bacc
import concourse.mybir as mybir
import concourse.tile as tile
from concourse.bass_utils import run_bass_kernel_spmd
from concourse.masks import make_identity

BF16 = mybir.dt.bfloat16
F32 = mybir.dt.float32
F16 = mybir.dt.float16
AF = mybir.ActivationFunctionType
ALU = mybir.AluOpType

V = 50257
VPAD = 51200          # 2 * 25600
VSH = VPAD // 2       # per-core vocab shard
D = 768
H = 12
HD = 64
L = 2
T = 1024
B = 4
EPS = 1e-5
NKT = D // 128        # 6 k-tiles over D
NQC = T // 512        # 2 q-chunks
NTT = T // 128        # 8 token-tiles
NVC = VSH // 512      # 50 lm vocab chunks per core

TRACE = False
LAST_RESULT = None

_SINGLES = {}


def _ln_phase(tc, nc, tag, xt, g_d, b_d, out_tiles, small, scratch):
    """LayerNorm over partition dim (features) of xt (6 fp32 [128,1024] tiles).
    g_d/b_d: [768] bf16 DRAM APs. Writes bf16 out_tiles (6 x [128,1024])."""
    ones_bf = _SINGLES["ones_bf"]
    ones_row = _SINGLES["ones_row"]

    g_bf = small.tile([1, D], BF16, tag="g_bf", name="g_bf")
    b_bf = small.tile([1, D], BF16, tag="b_bf", name="b_bf")
    nc.sync.dma_start(g_bf, g_d.rearrange("(o d) -> o d", o=1))
    nc.sync.dma_start(b_bf, b_d.rearrange("(o d) -> o d", o=1))
    rstd_bf = small.tile([1, T], BF16, tag="rstd_bf", name="rstd_bf")
    nmr_bf = small.tile([1, T], BF16, tag="nmr_bf", name="nmr_bf")
    eps_sb = small.tile([1, 1], F32, tag="eps_sb", name="eps_sb")
    nc.vector.memset(eps_sb, EPS)

    with tc.tile_pool(name=f"stps_{tag}", bufs=1, space="PSUM") as stats_ps, \
         tc.tile_pool(name=f"abps_{tag}", bufs=2, space="PSUM") as ab_ps:
        for c in range(NQC):
            s1 = stats_ps.tile([1, 512], F32, tag="s1", name="s1")
            s2 = stats_ps.tile([1, 512], F32, tag="s2", name="s2")
            for kt in range(NKT):
                xbf = scratch.tile([128, 512], BF16, tag="xbf", name="xbf")
                sq = scratch.tile([128, 512], BF16, tag="sq", name="sq")
                xs = xt[kt][:, c * 512:(c + 1) * 512]
                nc.vector.tensor_copy(xbf, xs)
                nc.vector.tensor_mul(sq, xs, xs)
                nc.tensor.matmul(s1, ones_bf, xbf,
                                 start=(kt == 0), stop=(kt == NKT - 1))
                nc.tensor.matmul(s2, ones_bf, sq,
                                 start=(kt == 0), stop=(kt == NKT - 1))
            # mean = s1/D ; var = s2/D - mean^2 ; rstd = 1/sqrt(var+eps)
            mean = small.tile([1, 512], F32, tag="mean", name="mean")
            var = small.tile([1, 512], F32, tag="var", name="var")
            rstd = small.tile([1, 512], F32, tag="rstd", name="rstd")
            nc.vector.tensor_scalar_mul(mean, s1, 1.0 / D)
            nc.vector.tensor_mul(var, mean, mean)
            nc.vector.scalar_tensor_tensor(var, s2, 1.0 / D, var,
                                           op0=ALU.mult, op1=ALU.subtract)
            nc.scalar.activation(var, var, AF.Sqrt, bias=eps_sb)
            nc.vector.reciprocal(rstd, var)
            nc.vector.tensor_copy(rstd_bf[:, c * 512:(c + 1) * 512], rstd)
            # nmr = -mean*rstd
            nc.vector.scalar_tensor_tensor(var, mean, -1.0, rstd,
                                           op0=ALU.mult, op1=ALU.mult)
            nc.vector.tensor_copy(nmr_bf[:, c * 512:(c + 1) * 512], var)

        for kt in range(NKT):
            gs = g_bf[0:1, kt * 128:(kt + 1) * 128]
            bs = b_bf[0:1, kt * 128:(kt + 1) * 128]
            for c in range(NQC):
                cs = slice(c * 512, (c + 1) * 512)
                a_ps = ab_ps.tile([128, 512], F32, tag="a_ps", name="a_ps")
                b_ps = ab_ps.tile([128, 512], F32, tag="b_ps", name="b_ps")
                nc.tensor.matmul(a_ps, gs, rstd_bf[:, cs], start=True, stop=True)
                nc.tensor.matmul(b_ps, gs, nmr_bf[:, cs], start=True, stop=False)
                nc.tensor.matmul(b_ps, bs, ones_row[:, 0:512],
                                 start=False, stop=True)
                tmp = scratch.tile([128, 512], F32, tag="lntmp", name="lntmp")
                nc.vector.tensor_mul(tmp, xt[kt][:, cs], a_ps)
                nc.vector.tensor_add(out_tiles[kt][:, cs], tmp, b_ps)


def build_bass():
    nc = bacc.Bacc(None, target_bir_lowering=False)
    # ---- DRAM I/O (per-core shard views) ----
    xT_d = nc.dram_tensor("xT", [D, T], F32, kind="ExternalInput")
    qkw_d = nc.dram_tensor("qkw", [L, D, 2 * D], BF16, kind="ExternalInput")
    vw_d = nc.dram_tensor("vw", [L, D, D], BF16, kind="ExternalInput")
    pw_d = nc.dram_tensor("pw", [L, D, D], BF16, kind="ExternalInput")
    fcw_d = nc.dram_tensor("fcw", [L, D, 4 * D], BF16, kind="ExternalInput")
    fc2w_d = nc.dram_tensor("fc2w", [L, 4 * D, D], BF16, kind="ExternalInput")
    qkb_d = nc.dram_tensor("qkb", [L, 2 * D], F32, kind="ExternalInput")
    vb_d = nc.dram_tensor("vb", [L, D], BF16, kind="ExternalInput")
    pb_d = nc.dram_tensor("pb", [L, D], F32, kind="ExternalInput")
    fcb_d = nc.dram_tensor("fcb", [L, 4 * D], F32, kind="ExternalInput")
    fc2b_d = nc.dram_tensor("fc2b", [L, D], F32, kind="ExternalInput")
    ln_d = nc.dram_tensor("lnp", [L, 4, D], BF16, kind="ExternalInput")  # g1,b1,g2,b2
    lnf_d = nc.dram_tensor("lnf", [2, D], BF16, kind="ExternalInput")
    mask_d = nc.dram_tensor("mask", [4, 128, 512], BF16, kind="ExternalInput")
    lmw_d = nc.dram_tensor("lmw", [D, VSH], BF16, kind="ExternalInput")
    out_d = nc.dram_tensor("out", [T, VSH], F16, kind="ExternalOutput")

    with tile.TileContext(nc) as tc, ExitStack() as octx:
        singles = octx.enter_context(tc.tile_pool(name="singles", bufs=1))
        resid = octx.enter_context(tc.tile_pool(name="resid", bufs=1))

        # constants
        ones_bf = singles.tile([128, 1], BF16)
        nc.vector.memset(ones_bf, 1.0)
        ones_row = singles.tile([1, 512], BF16)
        nc.vector.memset(ones_row, 1.0)
        ident = singles.tile([128, 128], BF16)
        make_identity(nc, ident)
        _SINGLES["ones_bf"] = ones_bf
        _SINGLES["ones_row"] = ones_row

        mask_sb = singles.tile([128, 4, 512], BF16)
        nc.sync.dma_start(mask_sb, mask_d.rearrange("j p q -> p j q"))

        # residual stream, fp32, resident
        xt = [resid.tile([128, T], F32, tag=f"xt{i}", name=f"xt{i}") for i in range(NKT)]
        for kt in range(NKT):
            nc.sync.dma_start(xt[kt], xT_d[kt * 128:(kt + 1) * 128, :])

        for l in range(L):
            with ExitStack() as lctx:
                lnpool = lctx.enter_context(tc.tile_pool(name=f"ln{l}", bufs=1))
                wpool = lctx.enter_context(tc.tile_pool(name=f"w{l}", bufs=3))
                biasp = lctx.enter_context(tc.tile_pool(name=f"bias{l}", bufs=1))
                small = lctx.enter_context(tc.tile_pool(name=f"small{l}", bufs=2))
                scratch = lctx.enter_context(tc.tile_pool(name=f"scr{l}", bufs=3))

                qkb_sb = biasp.tile([128, 12], F32)
                nc.sync.dma_start(qkb_sb, qkb_d[l].rearrange("(t p) -> p t", p=128))
                vbbf_sb = biasp.tile([1, D], BF16)
                nc.sync.dma_start(vbbf_sb, vb_d[l].rearrange("(o d) -> o d", o=1))
                pb_sb = biasp.tile([128, 6], F32)
                nc.sync.dma_start(pb_sb, pb_d[l].rearrange("(t p) -> p t", p=128))
                fcb_sb = biasp.tile([128, 24], F32)
                nc.sync.dma_start(fcb_sb, fcb_d[l].rearrange("(t p) -> p t", p=128))
                fc2b_sb = biasp.tile([128, 6], F32)
                nc.sync.dma_start(fc2b_sb, fc2b_d[l].rearrange("(t p) -> p t", p=128))

                # ---------- LN1 ----------
                h_bf = [lnpool.tile([128, T], BF16, tag=f"hbf{i}", name=f"hbf{i}")
                        for i in range(NKT)]
                _ln_phase(tc, nc, f"l{l}a", xt, ln_d[l][0], ln_d[l][1],
                          h_bf, small, scratch)

                # ---------- qkT = (qk_w).T @ h  [1536, 1024] bf16 ----------
                qk_sb = [lnpool.tile([128, T], BF16, tag=f"qk{i}", name=f"qk{i}")
                         for i in range(12)]
                with tc.tile_pool(name=f"qkps{l}", bufs=3, space="PSUM") as qkps:
                    for f in range(12):
                        wt = wpool.tile([128, NKT, 128], BF16, tag="qkw_t", name="qkw_t")
                        nc.sync.dma_start(
                            wt, qkw_d[l][:, f * 128:(f + 1) * 128]
                            .rearrange("(t p) f -> p t f", p=128))
                        for c in range(NQC):
                            cs = slice(c * 512, (c + 1) * 512)
                            ps = qkps.tile([128, 512], F32, tag="qkps", name="qkps")
                            for kt in range(NKT):
                                nc.tensor.matmul(ps, wt[:, kt, :], h_bf[kt][:, cs],
                                                 start=(kt == 0),
                                                 stop=(kt == NKT - 1))
                            nc.scalar.activation(qk_sb[f][:, cs], ps, AF.Identity,
                                                 bias=qkb_sb[:, f:f + 1])

                    # ---------- V natural [tokens, 12, 65] bf16 (aug ones) ------
                    v_aug = [lnpool.tile([128, 12, 65], BF16, tag=f"vaug{i}", name=f"vaug{i}")
                             for i in range(NTT)]
                    vw_sb = [wpool.tile([128, D], BF16, tag=f"vw{i}", name=f"vw{i}", bufs=1)
                             for i in range(NKT)]
                    for kt in range(NKT):
                        nc.sync.dma_start(vw_sb[kt],
                                          vw_d[l][kt * 128:(kt + 1) * 128, :])
                    for tt in range(NTT):
                        nc.vector.memset(v_aug[tt][:, :, 64:65], 1.0)
                        for vc in range(2):
                            vs = slice(vc * 384, (vc + 1) * 384)
                            ps = qkps.tile([128, 384], F32, tag="vps", name="vps")
                            for kt in range(NKT):
                                nc.tensor.matmul(
                                    ps, h_bf[kt][:, tt * 128:(tt + 1) * 128],
                                    vw_sb[kt][:, vs],
                                    start=(kt == 0), stop=False)
                            nc.tensor.matmul(ps, ones_row[:, 0:128],
                                             vbbf_sb[:, vs],
                                             start=False, stop=True)
                            nc.vector.tensor_copy(
                                v_aug[tt][:, vc * 6:(vc + 1) * 6, 0:64],
                                ps.rearrange("p (h d) -> p h d", d=64))

                # ---------- attention per head-pair ----------
                attoT = [lnpool.tile([128, T], BF16, tag=f"attoT{i}", name=f"attoT{i}")
                         for i in range(NKT)]
                with tc.tile_pool(name=f"sps{l}", bufs=2, space="PSUM") as sps, \
                     tc.tile_pool(name=f"ops{l}", bufs=1, space="PSUM") as ops, \
                     tc.tile_pool(name=f"tps{l}", bufs=1, space="PSUM") as tps, \
                     tc.tile_pool(name=f"attp{l}", bufs=1) as attp:
                    for pr in range(6):
                        attT = [[attp.tile([128, T], BF16, tag=f"attT{hh}_{kt}", name=f"attT{hh}_{kt}")
                                 for kt in range(NTT)] for hh in range(2)]
                        psT = tps.tile([128, T], BF16, tag="psT", name="psT")
                        for c in range(NQC):
                            cs = slice(c * 512, (c + 1) * 512)
                            nkt = 4 * (c + 1)
                            for kt in range(nkt):
                                ks = slice(kt * 128, (kt + 1) * 128)
                                pss = [None, None]
                                for hh in range(2):
                                    ps = sps.tile([128, 512], F32, tag=f"sps{hh}", name=f"sps{hh}")
                                    pss[hh] = ps
                                    hs = slice(hh * 64, hh * 64 + 64)
                                    nc.tensor.matmul(
                                        ps,
                                        qk_sb[6 + pr][hs, ks],   # kT [64,128]
                                        qk_sb[pr][hs, cs],       # qT [64,512]
                                        start=True, stop=True)
                                partial = (c == 0) or (kt >= 4)
                                for hh in range(2):
                                    dst = attT[hh][kt][:, cs]
                                    nc.scalar.activation(dst, pss[hh], AF.Exp,
                                                         scale=0.125)
                                    if partial:
                                        nc.vector.tensor_mul(
                                            dst, dst, mask_sb[:, kt % 4, :])
                        for hh in range(2):
                            h = 2 * pr + hh
                            for qt in range(NTT):
                                po = ops.tile([128, 65], F32, tag=f"ops{hh}", name=f"ops{hh}")
                                for kt in range(qt + 1):
                                    nc.tensor.matmul(
                                        po,
                                        attT[hh][kt][:, qt * 128:(qt + 1) * 128],
                                        v_aug[kt][:, h, :],
                                        start=(kt == 0), stop=(kt == qt))
                                r_sb = scratch.tile([128, 1], F32, tag="r_sb", name="r_sb")
                                ao = scratch.tile([128, 64], BF16, tag="ao", name="ao")
                                nc.vector.reciprocal(r_sb, po[:, 64:65])
                                nc.vector.tensor_scalar_mul(ao, po[:, 0:64], r_sb)
                                nc.tensor.transpose(
                                    psT[hh * 64:hh * 64 + 64,
                                        qt * 128:(qt + 1) * 128],
                                    ao, ident,
                                    tile_position=(0, hh * 64))
                        nc.vector.tensor_copy(attoT[pr], psT)

                # ---------- proj + residual ----------
                pw_sb = [wpool.tile([128, D], BF16, tag=f"pw{i}", name=f"pw{i}", bufs=1)
                         for i in range(NKT)]
                for kt in range(NKT):
                    nc.sync.dma_start(pw_sb[kt], pw_d[l][kt * 128:(kt + 1) * 128, :])
                with tc.tile_pool(name=f"pps{l}", bufs=4, space="PSUM") as pps:
                    for ot in range(NKT):
                        for c in range(NQC):
                            cs = slice(c * 512, (c + 1) * 512)
                            ps = pps.tile([128, 512], F32, tag="pps", name="pps")
                            for kt in range(NKT):
                                nc.tensor.matmul(
                                    ps, pw_sb[kt][:, ot * 128:(ot + 1) * 128],
                                    attoT[kt][:, cs],
                                    start=(kt == 0), stop=(kt == NKT - 1))
                            nc.vector.scalar_tensor_tensor(
                                xt[ot][:, cs], ps, pb_sb[:, ot:ot + 1],
                                xt[ot][:, cs], op0=ALU.add, op1=ALU.add)

                # ---------- LN2 + MLP (token-chunked hidden) ----------
                h2in = [lnpool.tile([128, T], BF16, tag=f"hbf{i}", name=f"hbf{i}")
                        for i in range(NKT)]
                _ln_phase(tc, nc, f"l{l}b", xt, ln_d[l][2], ln_d[l][3],
                          h2in, small, scratch)

                with tc.tile_pool(name=f"mlpps{l}", bufs=3, space="PSUM") as mlpps, \
                     tc.tile_pool(name=f"h2p{l}", bufs=1) as h2p:
                    for c in range(NQC):
                        cs = slice(c * 512, (c + 1) * 512)
                        h2c = [h2p.tile([128, 512], BF16, tag=f"h2c{f}", name=f"h2c{f}")
                               for f in range(24)]
                        for f in range(24):
                            wt = wpool.tile([128, NKT, 128], BF16, tag="fcw_t", name="fcw_t")
                            nc.sync.dma_start(
                                wt, fcw_d[l][:, f * 128:(f + 1) * 128]
                                .rearrange("(t p) f -> p t f", p=128))
                            ps = mlpps.tile([128, 512], F32, tag="fcps", name="fcps")
                            for kt in range(NKT):
                                nc.tensor.matmul(ps, wt[:, kt, :], h2in[kt][:, cs],
                                                 start=(kt == 0),
                                                 stop=(kt == NKT - 1))
                            nc.scalar.activation(h2c[f], ps, AF.Gelu_apprx_tanh,
                                                 bias=fcb_sb[:, f:f + 1])
                        for ot in range(NKT):
                            wt = wpool.tile([128, 24, 128], BF16, tag="fc2w_t", name="fc2w_t", bufs=2)
                            nc.sync.dma_start(
                                wt, fc2w_d[l][:, ot * 128:(ot + 1) * 128]
                                .rearrange("(t p) f -> p t f", p=128))
                            ps = mlpps.tile([128, 512], F32, tag="fc2ps", name="fc2ps")
                            for kt in range(24):
                                nc.tensor.matmul(ps, wt[:, kt, :], h2c[kt],
                                                 start=(kt == 0), stop=(kt == 23))
                            nc.vector.scalar_tensor_tensor(
                                xt[ot][:, cs], ps, fc2b_sb[:, ot:ot + 1],
                                xt[ot][:, cs], op0=ALU.add, op1=ALU.add)

        # ---------- final LN + lm_head ----------
        with ExitStack() as fctx:
            lnpool = fctx.enter_context(tc.tile_pool(name="lnfp", bufs=1))
            biasp = fctx.enter_context(tc.tile_pool(name="biasf", bufs=1))
            small = fctx.enter_context(tc.tile_pool(name="smallf", bufs=2))
            scratch = fctx.enter_context(tc.tile_pool(name="scrf", bufs=3))
            xf_bf = [lnpool.tile([128, T], BF16, tag=f"xf{i}", name=f"xf{i}") for i in range(NKT)]
            _ln_phase(tc, nc, "lf", xt, lnf_d[0], lnf_d[1],
                      xf_bf, small, scratch)

            with tc.tile_pool(name="lmw", bufs=3) as lmwp, \
                 tc.tile_pool(name="lmps", bufs=4, space="PSUM") as lmps, \
                 tc.tile_pool(name="lmev", bufs=4) as lmev:
                for vc in range(NVC):
                    wt = lmwp.tile([128, NKT, 512], BF16, tag="lmw_t", name="lmw_t")
                    nc.sync.dma_start(
                        wt, lmw_d[:, vc * 512:(vc + 1) * 512]
                        .rearrange("(t p) v -> p t v", p=128))
                    for tt in range(NTT):
                        ps = lmps.tile([128, 512], F32, tag="lmps", name="lmps")
                        for kt in range(NKT):
                            nc.tensor.matmul(
                                ps, xf_bf[kt][:, tt * 128:(tt + 1) * 128],
                                wt[:, kt, :],
                                start=(kt == 0), stop=(kt == NKT - 1))
                        ev = lmev.tile([128, 512], F16, tag="lmev", name="lmev")
                        if tt % 2 == 0:
                            nc.scalar.copy(ev, ps)
                        else:
                            nc.vector.tensor_copy(ev, ps)
                        nc.sync.dma_start(
                            out_d[tt * 128:(tt + 1) * 128,
                                  vc * 512:(vc + 1) * 512], ev)
    nc.finalize()
    return nc


_NC_CACHE = None


def _get_nc():
    global _NC_CACHE
    if _NC_CACHE is None:
        _NC_CACHE = build_bass()
    return _NC_CACHE


def make_in_maps(idx, layer_num, wte, wpe, ln1_g, ln1_b, attn_w, attn_b, proj_w,
                 proj_b, ln2_g, ln2_b, fc_w, fc_b, fc2_w, fc2_b, lnf_g, lnf_b, lm_w):
    bf = ml_dtypes.bfloat16
    idx = np.asarray(idx)
    f32 = np.float32
    wte = np.asarray(wte, f32)
    wpe = np.asarray(wpe, f32)
    x0 = wte[idx] + wpe[:T]                      # [B,T,D] fp32 host embedding

    qkw = np.ascontiguousarray(np.asarray(attn_w, f32)[:, :, :2 * D]).astype(bf)
    vw = np.ascontiguousarray(np.asarray(attn_w, f32)[:, :, 2 * D:]).astype(bf)
    pw = np.asarray(proj_w, f32).astype(bf)
    fcw = np.asarray(fc_w, f32).astype(bf)
    fc2w = np.asarray(fc2_w, f32).astype(bf)
    qkb = np.ascontiguousarray(np.asarray(attn_b, f32)[:, :2 * D])
    vb = np.ascontiguousarray(np.asarray(attn_b, f32)[:, 2 * D:]).astype(bf)
    lnp = np.stack([np.asarray(ln1_g, f32), np.asarray(ln1_b, f32),
                    np.asarray(ln2_g, f32), np.asarray(ln2_b, f32)], axis=1).astype(bf)
    lnf = np.stack([np.asarray(lnf_g, f32), np.asarray(lnf_b, f32)], axis=0).astype(bf)

    lmw_pad = np.zeros((D, VPAD), f32)
    lmw_pad[:, :V] = np.asarray(lm_w, f32)
    lmw_bf = lmw_pad.astype(bf)

    # causal mask blocks: mask[j][kk, qq] = (128*j + kk) <= qq
    jj = np.arange(4)[:, None, None] * 128 + np.arange(128)[None, :, None]
    qq = np.arange(512)[None, None, :]
    mask = (jj <= qq).astype(bf)

    in_maps = []
    for core in range(8):
        b = core // 2
        vs = (core % 2) * VSH
        in_maps.append(dict(
            xT=np.ascontiguousarray(x0[b].T),
            qkw=qkw, vw=vw, pw=pw, fcw=fcw, fc2w=fc2w,
            qkb=qkb, vb=vb, pb=np.asarray(proj_b, f32),
            fcb=np.asarray(fc_b, f32), fc2b=np.asarray(fc2_b, f32),
            lnp=lnp, lnf=lnf, mask=mask,
            lmw=np.ascontiguousarray(lmw_bf[:, vs:vs + VSH]),
        ))
    return in_maps


def kernel(**inputs):
    global LAST_RESULT
    in_maps = make_in_maps(**inputs)
    nc = _get_nc()
    res = run_bass_kernel_spmd(nc, in_maps, core_ids=list(range(8)), trace=TRACE)
    LAST_RESULT = res

    logits = np.empty((B, T, V), np.float32)
    for b in range(B):
        lo = res.results[2 * b]["out"].astype(np.float32)
        hi = res.results[2 * b + 1]["out"].astype(np.float32)
        logits[b, :, :VSH] = lo
        logits[b, :, VSH:] = hi[:, :V - VSH]
    return logits



# revision 5
# speedup vs baseline: 1.1221x; 1.1221x over previous
"""GPT-2-ish forward (B=4, T=1024, D=768, H=12, L=2, V=50257) on 8 trn2 cores.

Sharding: core pair (2b, 2b+1) sequence-parallel over batch b's tokens:
parity p owns interleaved 128-token chunks {p, p+2, p+4, p+6} (512 tokens).
Per layer each core LNs its own tokens, AllGathers h within the pair (two
256-token chunks, pipelined against QKV compute), computes K/V for all 1024
tokens and Q/attention/proj/MLP for its own 512. lm_head: own tokens x full
vocab (padded to 50688), so no final exchange is needed.

On-device layout: activations [features, tokens]; residual fp32 resident.
Attention: scores per (head, key-tile) with kT stationary; exp on Scalar
(unnormalized, causal mask multiplied after); att@V with V stationary
[128, 65] (ones column appended so the softmax denominator lands in psum
partition 64); normalization = reciprocal + gpsimd partition_broadcast +
vector multiply, emitting attoT directly in [hd, tokens] layout (no
transposes). LayerNorm: column sums via ones-vector matmuls, mean/rstd
broadcasts materialized as rank-1 matmuls in PSUM, apply fused as
sub / mult / two-scalar tensor_scalar on Vector. All matmuls bf16 with
fp32 PSUM; logits evicted f16 and upcast on host.
"""

import numpy as np
import ml_dtypes
from contextlib import ExitStack

import concourse.bass as bass
from concourse import bacc
import concourse.mybir as mybir
import concourse.tile as tile
from concourse.bass_utils import run_bass_kernel_spmd

BF16 = mybir.dt.bfloat16
F32 = mybir.dt.float32
F16 = mybir.dt.float16
AF = mybir.ActivationFunctionType
ALU = mybir.AluOpType

V = 50257
VPAD = 50688          # 99 * 512
D = 768
H = 12
HD = 64
L = 2
T = 1024
B = 4
TOWN = 512            # tokens owned per core
EPS = 1e-5
NKT = D // 128        # 6 feature tiles
NTT = T // 128        # 8 global token tiles
NOS = TOWN // 128     # 4 own token sub-chunks
NVC = VPAD // 512     # 99 lm vocab chunks
PAIRS = [[0, 1], [2, 3], [4, 5], [6, 7]]

TRACE = False
LAST_RESULT = None

_S = {}


def _ln_phase(tc, nc, tag, xt, g_col, b_col, hout, small, scratch, lnps):
    """LayerNorm over features (partition dim) of own tokens.
    xt: [128, NKT, TOWN] f32; g_col/b_col: [128, NKT] f32 per-feature params
    (column kt = features kt*128..). hout: [128, NKT, TOWN] bf16.
    Emits apply in two 256-column halves so callers can consume early."""
    ones_bf = _S["ones_bf"]
    ones_row = _S["ones_row"]
    eps_sb = _S["eps_sb"]

    s1 = lnps.tile([1, TOWN], F32, tag="s1", name="s1")
    s2 = lnps.tile([1, TOWN], F32, tag="s2", name="s2")
    for kt in range(NKT):
        xbf = scratch.tile([128, TOWN], BF16, tag="xbf", name="xbf")
        sq = scratch.tile([128, TOWN], BF16, tag="sq", name="sq")
        xs = xt[:, kt, :]
        nc.vector.tensor_copy(xbf, xs)
        nc.vector.tensor_mul(sq, xs, xs)
        nc.tensor.matmul(s1, ones_bf, xbf, start=(kt == 0), stop=(kt == NKT - 1))
        nc.tensor.matmul(s2, ones_bf, sq, start=(kt == 0), stop=(kt == NKT - 1))
    # mean = s1/D ; var = s2/D - mean^2 ; rstd = rsqrt(var+eps)
    mean = small.tile([1, TOWN], F32, tag="mean", name="mean")
    var = small.tile([1, TOWN], F32, tag="var", name="var")
    rstd = small.tile([1, TOWN], F32, tag="rstd", name="rstd")
    mean_bf = small.tile([1, TOWN], BF16, tag="mean_bf", name="mean_bf")
    rstd_bf = small.tile([1, TOWN], BF16, tag="rstd_bf", name="rstd_bf")
    nc.vector.tensor_scalar_mul(mean, s1, 1.0 / D)
    nc.vector.tensor_mul(var, mean, mean)
    nc.vector.scalar_tensor_tensor(var, s2, 1.0 / D, var,
                                   op0=ALU.mult, op1=ALU.subtract)
    nc.scalar.activation(var, var, AF.Sqrt, bias=eps_sb)
    nc.vector.reciprocal(rstd, var)
    nc.vector.tensor_copy(mean_bf, mean)
    nc.vector.tensor_copy(rstd_bf, rstd)
    # broadcast fields: mb = 1 (x) mean ; rb = 1 (x) rstd   [128, TOWN] psum
    mb = lnps.tile([128, TOWN], F32, tag="mb", name="mb")
    rb = lnps.tile([128, TOWN], F32, tag="rb", name="rb")
    nc.tensor.matmul(mb, ones_row[0:1, 0:128], mean_bf, start=True, stop=True)
    nc.tensor.matmul(rb, ones_row[0:1, 0:128], rstd_bf, start=True, stop=True)
    # apply: h = ((x - mb) * rb) * g + b, in column halves
    for ch in range(2):
        cs = slice(ch * 256, (ch + 1) * 256)
        for kt in range(NKT):
            tmp = scratch.tile([128, 256], F32, tag="lntmp", name="lntmp")
            nc.vector.tensor_sub(tmp, xt[:, kt, cs], mb[:, cs])
            nc.vector.tensor_mul(tmp, tmp, rb[:, cs])
            nc.vector.tensor_scalar(hout[:, kt, cs], tmp,
                                    g_col[:, kt:kt + 1], b_col[:, kt:kt + 1],
                                    op0=ALU.mult, op1=ALU.add)


def build_bass():
    nc = bacc.Bacc(None, target_bir_lowering=False)
    # ---- DRAM I/O (per-core) ----
    xT_d = nc.dram_tensor("xT", [D, TOWN], F32, kind="ExternalInput")
    qkw_d = nc.dram_tensor("qkw", [L, D, 2 * D], BF16, kind="ExternalInput")
    vw_d = nc.dram_tensor("vw", [L, D, D], BF16, kind="ExternalInput")
    pw_d = nc.dram_tensor("pw", [L, D, D], BF16, kind="ExternalInput")
    fcw_d = nc.dram_tensor("fcw", [L, D, 4 * D], BF16, kind="ExternalInput")
    fc2w_d = nc.dram_tensor("fc2w", [L, 4 * D, D], BF16, kind="ExternalInput")
    qkb_d = nc.dram_tensor("qkb", [L, 2 * D], F32, kind="ExternalInput")
    vb_d = nc.dram_tensor("vb", [L, D], BF16, kind="ExternalInput")
    pb_d = nc.dram_tensor("pb", [L, D], F32, kind="ExternalInput")
    fcb_d = nc.dram_tensor("fcb", [L, 4 * D], F32, kind="ExternalInput")
    fc2b_d = nc.dram_tensor("fc2b", [L, D], F32, kind="ExternalInput")
    ln_d = nc.dram_tensor("lnp", [L, 4, D], F32, kind="ExternalInput")  # g1,b1,g2,b2
    lnf_d = nc.dram_tensor("lnf", [2, D], F32, kind="ExternalInput")
    mask_d = nc.dram_tensor("mask", [NTT, 128, TOWN], BF16, kind="ExternalInput")
    lmw_d = nc.dram_tensor("lmw", [D, VPAD], BF16, kind="ExternalInput")
    out_d = nc.dram_tensor("out", [TOWN, VPAD], F16, kind="ExternalOutput")

    with tile.TileContext(nc) as tc, ExitStack() as octx:
        singles = octx.enter_context(tc.tile_pool(name="singles", bufs=1))
        resid = octx.enter_context(tc.tile_pool(name="resid", bufs=1))
        dram = octx.enter_context(tc.tile_pool(name="dram", bufs=2, space="DRAM"))

        ones_bf = singles.tile([128, 1], BF16)
        nc.vector.memset(ones_bf, 1.0)
        ones_row = singles.tile([1, 512], BF16)
        nc.vector.memset(ones_row, 1.0)
        eps_sb = singles.tile([1, 1], F32)
        nc.vector.memset(eps_sb, EPS)
        _S["ones_bf"] = ones_bf
        _S["ones_row"] = ones_row
        _S["eps_sb"] = eps_sb

        mask_sb = singles.tile([128, NTT, TOWN], BF16)
        nc.sync.dma_start(mask_sb, mask_d.rearrange("j p q -> p j q"))

        # residual stream (own tokens), fp32, resident
        xt = resid.tile([128, NKT, TOWN], F32)
        nc.sync.dma_start(xt, xT_d.rearrange("(k p) t -> p k t", p=128))

        for l in range(L):
            with ExitStack() as lctx:
                lnpool = lctx.enter_context(tc.tile_pool(name=f"ln{l}", bufs=1))
                wpool = lctx.enter_context(tc.tile_pool(name=f"w{l}", bufs=3))
                biasp = lctx.enter_context(tc.tile_pool(name=f"bias{l}", bufs=1))
                small = lctx.enter_context(tc.tile_pool(name=f"small{l}", bufs=2))
                scratch = lctx.enter_context(tc.tile_pool(name=f"scr{l}", bufs=3))

                qkb_sb = biasp.tile([128, 12], F32)
                nc.sync.dma_start(qkb_sb, qkb_d[l].rearrange("(t p) -> p t", p=128))
                vbbf_sb = biasp.tile([1, D], BF16)
                nc.sync.dma_start(vbbf_sb, vb_d[l].rearrange("(o d) -> o d", o=1))
                pb_sb = biasp.tile([128, 6], F32)
                nc.sync.dma_start(pb_sb, pb_d[l].rearrange("(t p) -> p t", p=128))
                fcb_sb = biasp.tile([128, 24], F32)
                nc.sync.dma_start(fcb_sb, fcb_d[l].rearrange("(t p) -> p t", p=128))
                fc2b_sb = biasp.tile([128, 6], F32)
                nc.sync.dma_start(fc2b_sb, fc2b_d[l].rearrange("(t p) -> p t", p=128))
                ln_sb = biasp.tile([128, 4, NKT], F32)
                nc.sync.dma_start(ln_sb, ln_d[l].rearrange("g (k p) -> p g k", p=128))

                # ---------- LN1 (own tokens) ----------
                h = lnpool.tile([128, NKT, TOWN], BF16, tag="h", name="h")
                with tc.tile_pool(name=f"lnps{l}a", bufs=1, space="PSUM") as lnps:
                    _ln_phase(tc, nc, f"l{l}a", xt, ln_sb[:, 0, :], ln_sb[:, 1, :],
                              h, small, scratch, lnps)

                # ---------- AllGather h within pair (2 token chunks) ------
                hfull = lnpool.tile([128, NKT, T], BF16, tag="hfull", name="hfull")
                for ch in range(2):
                    cs = slice(ch * 256, (ch + 1) * 256)
                    agin = dram.tile([D, 256], BF16, tag="agin", name="agin")
                    agout = dram.tile([2, D, 256], BF16, tag="agout", name="agout")
                    nc.sync.dma_start(
                        agin.rearrange("(k p) t -> p k t", p=128), h[:, :, cs])
                    nc.gpsimd.collective_compute(
                        "AllGather", ALU.bypass, replica_groups=PAIRS,
                        ins=[agin.opt()], outs=[agout.opt()])
                    # global chunk 4ch + 2s + r lands at block position s*256+r*128
                    hdst = hfull.rearrange("p k (c s r t) -> c r p k s t",
                                           c=2, s=2, r=2, t=128)
                    for r in range(2):
                        asrc = agout[r].rearrange("(k p) (s t) -> s p k t",
                                                  p=128, t=128)
                        for s in range(2):
                            nc.sync.dma_start(hdst[ch, r, :, :, s, :], asrc[s])

                # ---------- qT own [128, 6, TOWN] ----------
                q_sb = lnpool.tile([128, NKT, TOWN], BF16, tag="q_sb", name="q_sb")
                k_sb = lnpool.tile([128, NKT, T], BF16, tag="k_sb", name="k_sb")
                with tc.tile_pool(name=f"qkps{l}", bufs=3, space="PSUM") as qkps:
                    for f in range(NKT):
                        wt = wpool.tile([128, NKT, 128], BF16, tag="qw_t", name="qw_t")
                        nc.sync.dma_start(
                            wt, qkw_d[l][:, f * 128:(f + 1) * 128]
                            .rearrange("(t p) f -> p t f", p=128))
                        ps = qkps.tile([128, TOWN], F32, tag="qkps", name="qkps")
                        for kt in range(NKT):
                            nc.tensor.matmul(ps, wt[:, kt, :], h[:, kt, :],
                                             start=(kt == 0), stop=(kt == NKT - 1))
                        nc.vector.tensor_scalar_add(q_sb[:, f, :], ps,
                                                    qkb_sb[:, f:f + 1])

                    # ---------- kT full + V natural, per gathered chunk ----
                    v_aug = [lnpool.tile([128, 12, 65], BF16, tag=f"vaug{i}",
                                         name=f"vaug{i}") for i in range(NTT)]
                    vw_sb = [wpool.tile([128, D], BF16, tag=f"vw{i}",
                                        name=f"vw{i}", bufs=1) for i in range(NKT)]
                    for kt in range(NKT):
                        nc.sync.dma_start(vw_sb[kt],
                                          vw_d[l][kt * 128:(kt + 1) * 128, :])
                    for ch in range(2):
                        gs = slice(ch * 512, (ch + 1) * 512)
                        for f in range(NKT):
                            wt = wpool.tile([128, NKT, 128], BF16, tag="kw_t",
                                            name="kw_t")
                            nc.sync.dma_start(
                                wt, qkw_d[l][:, D + f * 128:D + (f + 1) * 128]
                                .rearrange("(t p) f -> p t f", p=128))
                            ps = qkps.tile([128, 512], F32, tag="qkps", name="qkps")
                            for kt in range(NKT):
                                nc.tensor.matmul(ps, wt[:, kt, :], hfull[:, kt, gs],
                                                 start=(kt == 0),
                                                 stop=(kt == NKT - 1))
                            nc.vector.tensor_scalar_add(k_sb[:, f, gs], ps,
                                                        qkb_sb[:, 6 + f:7 + f])
                        for tt in range(ch * 4, ch * 4 + 4):
                            nc.vector.memset(v_aug[tt][:, :, 64:65], 1.0)
                            for vc in range(2):
                                vs = slice(vc * 384, (vc + 1) * 384)
                                ps = qkps.tile([128, 384], F32, tag="vps", name="vps")
                                for kt in range(NKT):
                                    nc.tensor.matmul(
                                        ps, hfull[:, kt, tt * 128:(tt + 1) * 128],
                                        vw_sb[kt][:, vs],
                                        start=(kt == 0), stop=False)
                                nc.tensor.matmul(ps, ones_row[:, 0:128],
                                                 vbbf_sb[:, vs],
                                                 start=False, stop=True)
                                nc.vector.tensor_copy(
                                    v_aug[tt][:, vc * 6:(vc + 1) * 6, 0:64],
                                    ps.rearrange("p (h d) -> p h d", d=64))

                # ---------- attention per head-pair ----------
                attoT = lnpool.tile([128, NKT, TOWN], BF16, tag="attoT", name="attoT")
                with tc.tile_pool(name=f"sps{l}", bufs=3, space="PSUM") as sps, \
                     tc.tile_pool(name=f"ops{l}", bufs=2, space="PSUM") as ops, \
                     tc.tile_pool(name=f"attp{l}", bufs=3) as attp:
                    for pr in range(NKT):
                        attT = attp.tile([128, NTT, 2, TOWN], BF16,
                                         tag="attT", name="attT")
                        for kt in range(NTT):
                            for hh in range(2):
                                hs = slice(hh * 64, hh * 64 + 64)
                                ps = sps.tile([128, TOWN], F32, tag="sps", name="sps")
                                nc.tensor.matmul(
                                    ps,
                                    k_sb[hs, pr, kt * 128:(kt + 1) * 128],
                                    q_sb[hs, pr, :],
                                    start=True, stop=True)
                                dst = attT[:, kt, hh, :]
                                nc.scalar.activation(dst, ps, AF.Exp, scale=0.125)
                                nc.vector.tensor_mul(dst, dst, mask_sb[:, kt, :])
                        for hh in range(2):
                            hcol = 2 * pr + hh
                            po = ops.tile([65, TOWN], F32, tag=f"po{hh}",
                                          name=f"po{hh}")
                            for kt in range(NTT):
                                nc.tensor.matmul(
                                    po, v_aug[kt][:, hcol, :], attT[:, kt, hh, :],
                                    start=(kt == 0), stop=(kt == NTT - 1))
                            r_sb = scratch.tile([1, TOWN], F32, tag="r_sb",
                                                name="r_sb")
                            rbc = scratch.tile([64, TOWN], F32, tag="rbc",
                                               name="rbc")
                            nc.vector.reciprocal(r_sb, po[64:65, :])
                            nc.gpsimd.partition_broadcast(rbc, r_sb, channels=64)
                            nc.vector.tensor_mul(
                                attoT[hh * 64:hh * 64 + 64, pr, :],
                                po[0:64, :], rbc)

                # ---------- proj + residual ----------
                with tc.tile_pool(name=f"pps{l}", bufs=3, space="PSUM") as pps:
                    for ot in range(NKT):
                        wt = wpool.tile([128, NKT, 128], BF16, tag="pw_t",
                                        name="pw_t")
                        nc.sync.dma_start(
                            wt, pw_d[l][:, ot * 128:(ot + 1) * 128]
                            .rearrange("(t p) f -> p t f", p=128))
                        ps = pps.tile([128, TOWN], F32, tag="pps", name="pps")
                        for kt in range(NKT):
                            nc.tensor.matmul(ps, wt[:, kt, :], attoT[:, kt, :],
                                             start=(kt == 0), stop=(kt == NKT - 1))
                        nc.vector.scalar_tensor_tensor(
                            xt[:, ot, :], ps, pb_sb[:, ot:ot + 1],
                            xt[:, ot, :], op0=ALU.add, op1=ALU.add)

                # ---------- LN2 + MLP (own tokens) ----------
                h2 = lnpool.tile([128, NKT, TOWN], BF16, tag="h", name="h2")
                with tc.tile_pool(name=f"lnps{l}b", bufs=1, space="PSUM") as lnps:
                    _ln_phase(tc, nc, f"l{l}b", xt, ln_sb[:, 2, :], ln_sb[:, 3, :],
                              h2, small, scratch, lnps)

                with tc.tile_pool(name=f"mlpps{l}", bufs=3, space="PSUM") as mlpps, \
                     tc.tile_pool(name=f"h2p{l}", bufs=1) as h2p:
                    h2c = h2p.tile([128, 24, TOWN], BF16, tag="h2c", name="h2c")
                    for f in range(24):
                        wt = wpool.tile([128, NKT, 128], BF16, tag="fcw_t",
                                        name="fcw_t")
                        nc.sync.dma_start(
                            wt, fcw_d[l][:, f * 128:(f + 1) * 128]
                            .rearrange("(t p) f -> p t f", p=128))
                        ps = mlpps.tile([128, TOWN], F32, tag="fcps", name="fcps")
                        for kt in range(NKT):
                            nc.tensor.matmul(ps, wt[:, kt, :], h2[:, kt, :],
                                             start=(kt == 0), stop=(kt == NKT - 1))
                        nc.scalar.activation(h2c[:, f, :], ps, AF.Gelu_apprx_tanh,
                                             bias=fcb_sb[:, f:f + 1])
                    for ot in range(NKT):
                        wt = wpool.tile([128, 24, 128], BF16, tag="fc2w_t",
                                        name="fc2w_t", bufs=2)
                        nc.sync.dma_start(
                            wt, fc2w_d[l][:, ot * 128:(ot + 1) * 128]
                            .rearrange("(t p) f -> p t f", p=128))
                        ps = mlpps.tile([128, TOWN], F32, tag="fc2ps", name="fc2ps")
                        for kt in range(24):
                            nc.tensor.matmul(ps, wt[:, kt, :], h2c[:, kt, :],
                                             start=(kt == 0), stop=(kt == 23))
                        nc.vector.scalar_tensor_tensor(
                            xt[:, ot, :], ps, fc2b_sb[:, ot:ot + 1],
                            xt[:, ot, :], op0=ALU.add, op1=ALU.add)

        # ---------- final LN + lm_head (own tokens, full vocab) ----------
        with ExitStack() as fctx:
            lnpool = fctx.enter_context(tc.tile_pool(name="lnfp", bufs=1))
            biasp = fctx.enter_context(tc.tile_pool(name="biasf", bufs=1))
            small = fctx.enter_context(tc.tile_pool(name="smallf", bufs=2))
            scratch = fctx.enter_context(tc.tile_pool(name="scrf", bufs=3))
            lnf_sb = biasp.tile([128, 2, NKT], F32)
            nc.sync.dma_start(lnf_sb, lnf_d.rearrange("g (k p) -> p g k", p=128))
            xf = lnpool.tile([128, NKT, TOWN], BF16, tag="xf", name="xf")
            with tc.tile_pool(name="lnpsf", bufs=1, space="PSUM") as lnps:
                _ln_phase(tc, nc, "lf", xt, lnf_sb[:, 0, :], lnf_sb[:, 1, :],
                          xf, small, scratch, lnps)

            with tc.tile_pool(name="lmw", bufs=3) as lmwp, \
                 tc.tile_pool(name="lmps", bufs=4, space="PSUM") as lmps, \
                 tc.tile_pool(name="lmev", bufs=4) as lmev:
                for vc in range(NVC):
                    wt = lmwp.tile([128, NKT, 512], BF16, tag="lmw_t", name="lmw_t")
                    nc.sync.dma_start(
                        wt, lmw_d[:, vc * 512:(vc + 1) * 512]
                        .rearrange("(t p) v -> p t v", p=128))
                    for tt in range(NOS):
                        ps = lmps.tile([128, 512], F32, tag="lmps", name="lmps")
                        for kt in range(NKT):
                            nc.tensor.matmul(
                                ps, xf[:, kt, tt * 128:(tt + 1) * 128],
                                wt[:, kt, :],
                                start=(kt == 0), stop=(kt == NKT - 1))
                        ev = lmev.tile([128, 512], F16, tag="lmev", name="lmev")
                        if tt % 2 == 0:
                            nc.scalar.copy(ev, ps)
                        else:
                            nc.vector.tensor_copy(ev, ps)
                        nc.sync.dma_start(
                            out_d[tt * 128:(tt + 1) * 128,
                                  vc * 512:(vc + 1) * 512], ev)
    nc.finalize()
    return nc


_NC_CACHE = None


def _get_nc():
    global _NC_CACHE
    if _NC_CACHE is None:
        _NC_CACHE = build_bass()
    return _NC_CACHE


def make_in_maps(idx, layer_num, wte, wpe, ln1_g, ln1_b, attn_w, attn_b, proj_w,
                 proj_b, ln2_g, ln2_b, fc_w, fc_b, fc2_w, fc2_b, lnf_g, lnf_b, lm_w):
    bf = ml_dtypes.bfloat16
    idx = np.asarray(idx)
    f32 = np.float32
    wte = np.asarray(wte, f32)
    wpe = np.asarray(wpe, f32)
    x0 = wte[idx] + wpe[:T]                      # [B,T,D] fp32 host embedding

    qkw = np.ascontiguousarray(np.asarray(attn_w, f32)[:, :, :2 * D]).astype(bf)
    vw = np.ascontiguousarray(np.asarray(attn_w, f32)[:, :, 2 * D:]).astype(bf)
    pw = np.asarray(proj_w, f32).astype(bf)
    fcw = np.asarray(fc_w, f32).astype(bf)
    fc2w = np.asarray(fc2_w, f32).astype(bf)
    qkb = np.ascontiguousarray(np.asarray(attn_b, f32)[:, :2 * D])
    vb = np.ascontiguousarray(np.asarray(attn_b, f32)[:, 2 * D:]).astype(bf)
    lnp = np.stack([np.asarray(ln1_g, f32), np.asarray(ln1_b, f32),
                    np.asarray(ln2_g, f32), np.asarray(ln2_b, f32)],
                   axis=1)                        # [L, 4, D] f32
    lnf = np.stack([np.asarray(lnf_g, f32), np.asarray(lnf_b, f32)], axis=0)

    lmw_pad = np.zeros((D, VPAD), f32)
    lmw_pad[:, :V] = np.asarray(lm_w, f32)
    lmw_bf = lmw_pad.astype(bf)

    in_maps = []
    for core in range(8):
        b = core // 2
        p = core % 2
        own = np.concatenate([np.arange(128) + (2 * s + p) * 128
                              for s in range(NOS)])          # own global tokens
        # mask[kt][kp, j] = (kt*128 + kp) <= own[j]
        kglob = np.arange(NTT)[:, None, None] * 128 + np.arange(128)[None, :, None]
        mask = (kglob <= own[None, None, :]).astype(bf)      # [8, 128, 512]
        in_maps.append(dict(
            xT=np.ascontiguousarray(x0[b].T[:, own]),
            qkw=qkw, vw=vw, pw=pw, fcw=fcw, fc2w=fc2w,
            qkb=qkb, vb=vb, pb=np.asarray(proj_b, f32),
            fcb=np.asarray(fc_b, f32), fc2b=np.asarray(fc2_b, f32),
            lnp=lnp, lnf=lnf, mask=mask,
            lmw=lmw_bf,
        ))
    return in_maps


def kernel(**inputs):
    global LAST_RESULT
    in_maps = make_in_maps(**inputs)
    nc = _get_nc()
    res = run_bass_kernel_spmd(nc, in_maps, core_ids=list(range(8)), trace=TRACE)
    LAST_RESULT = res

    logits = np.empty((B, T, V), np.float32)
    for core in range(8):
        b = core // 2
        p = core % 2
        o = res.results[core]["out"].astype(np.float32)      # [TOWN, VPAD]
        for s in range(NOS):
            g = 2 * s + p
            logits[b, g * 128:(g + 1) * 128, :] = o[s * 128:(s + 1) * 128, :V]
    return logits


# revision 20
# speedup vs baseline: 1.2241x; 1.0909x over previous
"""GPT-2-ish forward (B=4, T=1024, D=768, H=12, L=2, V=50257) on 8 trn2 cores.

Sharding: core pair (2b, 2b+1) sequence-parallel over batch b's tokens:
parity p owns interleaved 128-token chunks {p, p+2, p+4, p+6} (512 tokens).
Per layer each core LNs its own tokens, AllGathers h within the pair (two
256-token chunks, pipelined against QKV compute), computes K/V for all 1024
tokens and Q/attention/proj/MLP for its own 512. lm_head: own tokens x full
vocab (padded to 50688), so no final exchange is needed.

On-device layout: activations [features, tokens]; residual fp32 resident.
Attention: scores per (head, key-tile) with kT stationary; exp on Scalar
(unnormalized, causal mask multiplied after); att@V with V stationary
[128, 65] (ones column appended so the softmax denominator lands in psum
partition 64); normalization = reciprocal + gpsimd partition_broadcast +
vector multiply, emitting attoT directly in [hd, tokens] layout (no
transposes). LayerNorm: column sums via ones-vector matmuls, mean/rstd
broadcasts materialized as rank-1 matmuls in PSUM, apply fused as
sub / mult / two-scalar tensor_scalar on Vector. All matmuls bf16 with
fp32 PSUM; logits evicted f16 and upcast on host.
"""

import numpy as np
import ml_dtypes
from contextlib import ExitStack

import concourse.bass as bass
from concourse import bacc
import concourse.mybir as mybir
import concourse.tile as tile
from concourse.bass_utils import run_bass_kernel_spmd

BF16 = mybir.dt.bfloat16
F32 = mybir.dt.float32
F16 = mybir.dt.float16
AF = mybir.ActivationFunctionType
ALU = mybir.AluOpType

V = 50257
VPAD = 50688          # 99 * 512
D = 768
H = 12
HD = 64
L = 2
T = 1024
B = 4
TOWN = 512            # tokens owned per core
EPS = 1e-5
NKT = D // 128        # 6 feature tiles
NTT = T // 128        # 8 global token tiles
NOS = TOWN // 128     # 4 own token sub-chunks
NVC = VPAD // 512     # 99 lm vocab chunks
PAIRS = [[0, 1], [2, 3], [4, 5], [6, 7]]

TRACE = False
LAST_RESULT = None

_S = {}


def _ln_phase(tc, nc, tag, xt, g_col, b_col, hout, small, scratch, lnps,
              post_half=None):
    """LayerNorm over features (partition dim) of own tokens, processed in
    two 256-token column halves so consumers (the AllGather) start early.
    xt: [128, NKT, TOWN] f32; g_col/b_col: [128, NKT] f32 per-feature params.
    hout: [128, NKT, TOWN] bf16. post_half(ch) is called after half ch."""
    ones_bf = _S["ones_bf"]
    ones_row = _S["ones_row"]
    eps_sb = _S["eps_sb"]

    for ch in range(2):
        cs = slice(ch * 256, (ch + 1) * 256)
        # full-bank padding: s1/s2 accumulation groups interleave, and a
        # start=True clears has_written bank-wide — they must not share one
        s1 = lnps.tile([1, 256], F32, tag="s1", name="s1",
                       padded_shape=[128, 512])
        s2 = lnps.tile([1, 256], F32, tag="s2", name="s2",
                       padded_shape=[128, 512])
        for kt in range(NKT):
            xbf = scratch.tile([128, 256], BF16, tag="xbf", name="xbf")
            sq = scratch.tile([128, 256], BF16, tag="sq", name="sq")
            xs = xt[:, kt, cs]
            nc.vector.tensor_copy(xbf, xs)
            nc.vector.tensor_mul(sq, xs, xs)
            nc.tensor.matmul(s1, ones_bf, xbf, start=(kt == 0),
                             stop=(kt == NKT - 1))
            nc.tensor.matmul(s2, ones_bf, sq, start=(kt == 0),
                             stop=(kt == NKT - 1))
        # mean = s1/D ; var = s2/D - mean^2 ; rstd = 1/sqrt(var+eps)
        mean = small.tile([1, 256], F32, tag="mean", name="mean")
        var = small.tile([1, 256], F32, tag="var", name="var")
        rstd = small.tile([1, 256], F32, tag="rstd", name="rstd")
        mean_bf = small.tile([1, 256], BF16, tag="mean_bf", name="mean_bf")
        rstd_bf = small.tile([1, 256], BF16, tag="rstd_bf", name="rstd_bf")
        nc.vector.tensor_scalar_mul(mean, s1, 1.0 / D)
        nc.vector.tensor_mul(var, mean, mean)
        nc.vector.scalar_tensor_tensor(var, s2, 1.0 / D, var,
                                       op0=ALU.mult, op1=ALU.subtract)
        nc.scalar.activation(var, var, AF.Sqrt, bias=eps_sb)
        nc.vector.reciprocal(rstd, var)
        nc.vector.tensor_copy(mean_bf, mean)
        nc.vector.tensor_copy(rstd_bf, rstd)
        # broadcast fields mb = 1 (x) mean, rb = 1 (x) rstd  [128, 256] psum
        mb = lnps.tile([128, 256], F32, tag="mb", name="mb")
        rb = lnps.tile([128, 256], F32, tag="rb", name="rb")
        nc.tensor.matmul(mb, ones_row[0:1, 0:128], mean_bf, start=True,
                         stop=True)
        nc.tensor.matmul(rb, ones_row[0:1, 0:128], rstd_bf, start=True,
                         stop=True)
        # apply: h = ((x - mb) * rb) * g + b
        for kt in range(NKT):
            tmp = scratch.tile([128, 256], F32, tag="lntmp", name="lntmp")
            nc.vector.tensor_sub(tmp, xt[:, kt, cs], mb)
            nc.vector.tensor_mul(tmp, tmp, rb)
            nc.vector.tensor_scalar(hout[:, kt, cs], tmp,
                                    g_col[:, kt:kt + 1], b_col[:, kt:kt + 1],
                                    op0=ALU.mult, op1=ALU.add)
        if post_half is not None:
            post_half(ch)


def build_bass():
    nc = bacc.Bacc(None, target_bir_lowering=False)
    # ---- DRAM I/O (per-core) ----
    xT_d = nc.dram_tensor("xT", [D, TOWN], F32, kind="ExternalInput")
    qkw_d = nc.dram_tensor("qkw", [L, D, 2 * D], BF16, kind="ExternalInput")
    vw_d = nc.dram_tensor("vw", [L, D, D], BF16, kind="ExternalInput")
    pw_d = nc.dram_tensor("pw", [L, D, D], BF16, kind="ExternalInput")
    fcw_d = nc.dram_tensor("fcw", [L, D, 4 * D], BF16, kind="ExternalInput")
    fc2w_d = nc.dram_tensor("fc2w", [L, 4 * D, D], BF16, kind="ExternalInput")
    qkb_d = nc.dram_tensor("qkb", [L, 2 * D], F32, kind="ExternalInput")
    vb_d = nc.dram_tensor("vb", [L, D], BF16, kind="ExternalInput")
    pb_d = nc.dram_tensor("pb", [L, D], F32, kind="ExternalInput")
    fcb_d = nc.dram_tensor("fcb", [L, 4 * D], F32, kind="ExternalInput")
    fc2b_d = nc.dram_tensor("fc2b", [L, D], F32, kind="ExternalInput")
    ln_d = nc.dram_tensor("lnp", [L, 4, D], F32, kind="ExternalInput")  # g1,b1,g2,b2
    lnf_d = nc.dram_tensor("lnf", [2, D], F32, kind="ExternalInput")
    mask_d = nc.dram_tensor("mask", [128, 256], BF16, kind="ExternalInput")
    lmw_d = nc.dram_tensor("lmw", [D, VPAD], BF16, kind="ExternalInput")
    out_d = nc.dram_tensor("out", [TOWN, VPAD], F16, kind="ExternalOutput")

    with tile.TileContext(nc) as tc, ExitStack() as octx:
        singles = octx.enter_context(tc.tile_pool(name="singles", bufs=1))
        resid = octx.enter_context(tc.tile_pool(name="resid", bufs=1))
        dram = octx.enter_context(tc.tile_pool(name="dram", bufs=2, space="DRAM"))

        ones_bf = singles.tile([128, 1], BF16)
        nc.vector.memset(ones_bf, 1.0)
        ones_row = singles.tile([1, 512], BF16)
        nc.vector.memset(ones_row, 1.0)
        eps_sb = singles.tile([1, 1], F32)
        nc.vector.memset(eps_sb, EPS)
        _S["ones_bf"] = ones_bf
        _S["ones_row"] = ones_row
        _S["eps_sb"] = eps_sb

        # mask for the last two key tiles of any strip s: cols 0:128 = tile
        # kt=2s (p=0: lower-tri, p=1: ones), cols 128:256 = tile kt=2s+1
        # (p=0: zeros, p=1: lower-tri)
        mask_sb = singles.tile([128, 256], BF16)
        nc.sync.dma_start(mask_sb, mask_d[:, :])

        # residual stream (own tokens), fp32, resident
        xt = resid.tile([128, NKT, TOWN], F32)
        nc.sync.dma_start(xt, xT_d.rearrange("(k p) t -> p k t", p=128))

        for l in range(L):
            with ExitStack() as lctx:
                lnpool = lctx.enter_context(tc.tile_pool(name=f"ln{l}", bufs=1))
                wpool = lctx.enter_context(tc.tile_pool(name=f"w{l}", bufs=3))
                biasp = lctx.enter_context(tc.tile_pool(name=f"bias{l}", bufs=1))
                small = lctx.enter_context(tc.tile_pool(name=f"small{l}", bufs=2))
                scratch = lctx.enter_context(tc.tile_pool(name=f"scr{l}", bufs=3))

                qkb_sb = biasp.tile([128, 12], F32)
                nc.sync.dma_start(qkb_sb, qkb_d[l].rearrange("(t p) -> p t", p=128))
                vbbf_sb = biasp.tile([1, D], BF16)
                nc.sync.dma_start(vbbf_sb, vb_d[l].rearrange("(o d) -> o d", o=1))
                pb_sb = biasp.tile([128, 6], F32)
                nc.sync.dma_start(pb_sb, pb_d[l].rearrange("(t p) -> p t", p=128))
                fcb_sb = biasp.tile([128, 24], F32)
                nc.sync.dma_start(fcb_sb, fcb_d[l].rearrange("(t p) -> p t", p=128))
                fc2b_sb = biasp.tile([128, 6], F32)
                nc.sync.dma_start(fc2b_sb, fc2b_d[l].rearrange("(t p) -> p t", p=128))
                ln_sb = biasp.tile([128, 4, NKT], F32)
                nc.sync.dma_start(ln_sb, ln_d[l].rearrange("g (k p) -> p g k", p=128))

                # ---------- LN1 (own tokens) + AllGather h within pair ----
                h = lnpool.tile([128, NKT, TOWN], BF16, tag="h", name="h")
                hfull = lnpool.tile([128, NKT, T], BF16, tag="hfull", name="hfull")

                def kick_ag(ch):
                    cs = slice(ch * 256, (ch + 1) * 256)
                    agin = dram.tile([D, 256], BF16, tag="agin", name="agin")
                    agout = dram.tile([2, D, 256], BF16, tag="agout", name="agout")
                    nc.sync.dma_start(
                        agin.rearrange("(k p) t -> p k t", p=128), h[:, :, cs])
                    nc.gpsimd.collective_compute(
                        "AllGather", ALU.bypass, replica_groups=PAIRS,
                        ins=[agin.opt()], outs=[agout.opt()])
                    # global chunk 4ch + 2s + r lands at block position s*256+r*128
                    hdst = hfull.rearrange("p k (c s r t) -> c r p k s t",
                                           c=2, s=2, r=2, t=128)
                    for r in range(2):
                        asrc = agout[r].rearrange("(k p) (s t) -> s p k t",
                                                  p=128, t=128)
                        for s in range(2):
                            nc.sync.dma_start(hdst[ch, r, :, :, s, :], asrc[s])

                with tc.tile_pool(name=f"lnps{l}a", bufs=1, space="PSUM") as lnps:
                    _ln_phase(tc, nc, f"l{l}a", xt, ln_sb[:, 0, :], ln_sb[:, 1, :],
                              h, small, scratch, lnps, post_half=kick_ag)

                # ---------- qT own, kT/V full, early scores --------------
                # Attention strips: per (pr, hh, s) the scores for own
                # q-sub-chunk s (128 cols) cover key tiles kt=0..2s+1 and are
                # packed [128, (2s+2)*128] in psum; exp evicts to bf16 strips.
                # s=0,1 (kt<=3, needs only AG chunk 0) run for all pr between
                # kT chunk 0 and kT chunk 1 to cover the AllGather latency.
                q_sb = lnpool.tile([128, NKT, TOWN], BF16, tag="q_sb", name="q_sb")
                k_sb = lnpool.tile([128, NKT, T], BF16, tag="k_sb", name="k_sb")
                attT01 = lnpool.tile([128, NKT, 2, 768], BF16, tag="attT01",
                                     name="attT01")
                v_aug = [lnpool.tile([128, 12, 65], BF16, tag=f"vaug{i}",
                                     name=f"vaug{i}") for i in range(NTT)]
                fcw_sb = wpool.tile([128, NKT, 4 * D], BF16, tag="fcw_l",
                                    name="fcw_sb", bufs=1)

                def scores_strip(sps_pool, stag, twid, pr, hh, s, dst):
                    """Scores+exp+mask for strip (pr, hh, s) -> dst bf16 AP."""
                    nk = 2 * s + 2
                    hs = slice(hh * 64, hh * 64 + 64)
                    st = sps_pool.tile([128, twid], F32, tag=stag, name=stag)
                    for kt in range(nk):
                        nc.tensor.matmul(
                            st[:, kt * 128:(kt + 1) * 128],
                            k_sb[hs, pr, kt * 128:(kt + 1) * 128],
                            q_sb[hs, pr, s * 128:(s + 1) * 128],
                            start=True, stop=True)
                    for a0 in range(0, nk * 128, 512):
                        a1 = min(nk * 128, a0 + 512)
                        nc.scalar.activation(dst[:, a0:a1], st[:, a0:a1],
                                             AF.Exp, scale=0.125)
                    # only the last two key tiles (kt=2s, 2s+1) need masking
                    nc.vector.tensor_mul(dst[:, (nk - 2) * 128:nk * 128],
                                         dst[:, (nk - 2) * 128:nk * 128],
                                         mask_sb)

                def kv_chunk(qkps, ch):
                    gs = slice(ch * 512, (ch + 1) * 512)
                    for f in range(NKT):
                        wt = wpool.tile([128, NKT, 128], BF16, tag="kw_t",
                                        name="kw_t", bufs=2)
                        nc.sync.dma_start(
                            wt, qkw_d[l][:, D + f * 128:D + (f + 1) * 128]
                            .rearrange("(t p) f -> p t f", p=128))
                        ps = qkps.tile([128, 512], F32, tag="qkps", name="qkps")
                        for kt in range(NKT):
                            nc.tensor.matmul(ps, wt[:, kt, :], hfull[:, kt, gs],
                                             start=(kt == 0),
                                             stop=(kt == NKT - 1))
                        nc.vector.tensor_scalar_add(k_sb[:, f, gs], ps,
                                                    qkb_sb[:, 6 + f:7 + f])
                    for tt in range(ch * 4, ch * 4 + 4):
                        nc.vector.memset(v_aug[tt][:, :, 64:65], 1.0)
                        for vc in range(2):
                            vs = slice(vc * 384, (vc + 1) * 384)
                            ps = qkps.tile([128, 384], F32, tag="vps", name="vps",
                                           bufs=2)
                            for kt in range(NKT):
                                nc.tensor.matmul(
                                    ps, hfull[:, kt, tt * 128:(tt + 1) * 128],
                                    vw_sb[kt][:, vs],
                                    start=(kt == 0), stop=False)
                            nc.tensor.matmul(ps, ones_row[:, 0:128],
                                             vbbf_sb[:, vs],
                                             start=False, stop=True)
                            nc.vector.tensor_copy(
                                v_aug[tt][:, vc * 6:(vc + 1) * 6, 0:64],
                                ps.rearrange("p (h d) -> p h d", d=64))

                with tc.tile_pool(name=f"qkps{l}", bufs=3, space="PSUM") as qkps:
                    for f in range(NKT):
                        wt = wpool.tile([128, NKT, 128], BF16, tag="qw_t",
                                        name="qw_t", bufs=2)
                        nc.sync.dma_start(
                            wt, qkw_d[l][:, f * 128:(f + 1) * 128]
                            .rearrange("(t p) f -> p t f", p=128))
                        ps = qkps.tile([128, TOWN], F32, tag="qkps", name="qkps")
                        for kt in range(NKT):
                            nc.tensor.matmul(ps, wt[:, kt, :], h[:, kt, :],
                                             start=(kt == 0), stop=(kt == NKT - 1))
                        nc.vector.tensor_scalar_add(q_sb[:, f, :], ps,
                                                    qkb_sb[:, f:f + 1])

                    vw_sb = [wpool.tile([128, D], BF16, tag=f"vw{i}",
                                        name=f"vw{i}", bufs=1) for i in range(NKT)]
                    for kt in range(NKT):
                        nc.sync.dma_start(vw_sb[kt],
                                          vw_d[l][kt * 128:(kt + 1) * 128, :])
                    kv_chunk(qkps, 0)
                    # early scores s=0,1 for all pr (covers AG chunk 1 wait)
                    nc.sync.dma_start(fcw_sb, fcw_d[l]
                                      .rearrange("(t p) f -> p t f", p=128))
                    for pr in range(NKT):
                        for hh in range(2):
                            for s in range(2):
                                scores_strip(
                                    qkps, "stripA", TOWN, pr, hh, s,
                                    attT01[:, pr, hh,
                                           s * 256:s * 256 + (2 * s + 2) * 128])
                    kv_chunk(qkps, 1)

                # ---------- attention tail per head-pair ------------------
                attoT = lnpool.tile([128, NKT, TOWN], BF16, tag="attoT",
                                    name="attoT")
                with tc.tile_pool(name=f"sps{l}", bufs=2, space="PSUM") as sps, \
                     tc.tile_pool(name=f"ops{l}", bufs=2, space="PSUM") as ops, \
                     tc.tile_pool(name=f"attp{l}", bufs=2) as attp:
                    for pr in range(NKT):
                        attT23 = attp.tile([128, 2, 1792], BF16, tag="attT23",
                                           name="attT23")
                        for hh in range(2):
                            for s in range(2, 4):
                                scores_strip(
                                    sps, "stripB", 1024, pr, hh, s,
                                    attT23[:, hh, (s - 2) * 768:
                                           (s - 2) * 768 + (2 * s + 2) * 128])
                        for hh in range(2):
                            hcol = 2 * pr + hh

                            def att_src(s, kt):
                                if s < 2:
                                    base = s * 256
                                    return attT01[:, pr, hh,
                                                  base + kt * 128:
                                                  base + (kt + 1) * 128]
                                base = (s - 2) * 768
                                return attT23[:, hh, base + kt * 128:
                                              base + (kt + 1) * 128]

                            po = ops.tile([65, TOWN], F32, tag=f"po{hh}",
                                          name=f"po{hh}")
                            # start=True clears has_written for the whole
                            # bank, so only the first MM opens the group;
                            # later regions overwrite-on-first-touch.
                            for kt in range(NTT):
                                for s in range(kt // 2, 4):
                                    nc.tensor.matmul(
                                        po[:, s * 128:(s + 1) * 128],
                                        v_aug[kt][:, hcol, :], att_src(s, kt),
                                        start=(kt == 0 and s == 0),
                                        stop=(kt == 2 * s + 1),
                                        skip_group_check=True)
                            r_sb = scratch.tile([1, TOWN], F32, tag="r_sb",
                                                name="r_sb")
                            rbc = scratch.tile([64, TOWN], F32, tag="rbc",
                                               name="rbc")
                            nc.vector.reciprocal(r_sb, po[64:65, :])
                            nc.gpsimd.partition_broadcast(rbc, r_sb, channels=64)
                            nc.vector.tensor_mul(
                                attoT[hh * 64:hh * 64 + 64, pr, :],
                                po[0:64, :], rbc)

                # ---------- proj + residual ----------
                with tc.tile_pool(name=f"pps{l}", bufs=3, space="PSUM") as pps:
                    for ot in range(NKT):
                        wt = wpool.tile([128, NKT, 128], BF16, tag="pw_t",
                                        name="pw_t")
                        nc.sync.dma_start(
                            wt, pw_d[l][:, ot * 128:(ot + 1) * 128]
                            .rearrange("(t p) f -> p t f", p=128))
                        ps = pps.tile([128, TOWN], F32, tag="pps", name="pps")
                        for kt in range(NKT):
                            nc.tensor.matmul(ps, wt[:, kt, :], attoT[:, kt, :],
                                             start=(kt == 0), stop=(kt == NKT - 1))
                        nc.vector.scalar_tensor_tensor(
                            xt[:, ot, :], ps, pb_sb[:, ot:ot + 1],
                            xt[:, ot, :], op0=ALU.add, op1=ALU.add)

                # ---------- LN2 + MLP (own tokens) ----------
                h2 = lnpool.tile([128, NKT, TOWN], BF16, tag="h", name="h2")
                with tc.tile_pool(name=f"lnps{l}b", bufs=1, space="PSUM") as lnps:
                    _ln_phase(tc, nc, f"l{l}b", xt, ln_sb[:, 2, :], ln_sb[:, 3, :],
                              h2, small, scratch, lnps)

                with tc.tile_pool(name=f"mlpps{l}", bufs=3, space="PSUM") as mlpps, \
                     tc.tile_pool(name=f"h2p{l}", bufs=1) as h2p:
                    h2c = h2p.tile([128, 24, TOWN], BF16, tag="h2c", name="h2c")
                    for f in range(24):
                        ps = mlpps.tile([128, TOWN], F32, tag="fcps", name="fcps")
                        for kt in range(NKT):
                            nc.tensor.matmul(ps, fcw_sb[:, kt, f * 128:(f + 1) * 128],
                                             h2[:, kt, :],
                                             start=(kt == 0), stop=(kt == NKT - 1))
                        nc.scalar.activation(h2c[:, f, :], ps, AF.Gelu_apprx_tanh,
                                             bias=fcb_sb[:, f:f + 1])
                    for ot in range(NKT):
                        wt = wpool.tile([128, 24, 128], BF16, tag="fc2w_t",
                                        name="fc2w_t", bufs=2)
                        nc.sync.dma_start(
                            wt, fc2w_d[l][:, ot * 128:(ot + 1) * 128]
                            .rearrange("(t p) f -> p t f", p=128))
                        ps = mlpps.tile([128, TOWN], F32, tag="fc2ps", name="fc2ps")
                        for kt in range(24):
                            nc.tensor.matmul(ps, wt[:, kt, :], h2c[:, kt, :],
                                             start=(kt == 0), stop=(kt == 23))
                        nc.vector.scalar_tensor_tensor(
                            xt[:, ot, :], ps, fc2b_sb[:, ot:ot + 1],
                            xt[:, ot, :], op0=ALU.add, op1=ALU.add)

        # ---------- final LN + lm_head (own tokens, full vocab) ----------
        with ExitStack() as fctx:
            lnpool = fctx.enter_context(tc.tile_pool(name="lnfp", bufs=1))
            biasp = fctx.enter_context(tc.tile_pool(name="biasf", bufs=1))
            small = fctx.enter_context(tc.tile_pool(name="smallf", bufs=2))
            scratch = fctx.enter_context(tc.tile_pool(name="scrf", bufs=3))
            lnf_sb = biasp.tile([128, 2, NKT], F32)
            nc.sync.dma_start(lnf_sb, lnf_d.rearrange("g (k p) -> p g k", p=128))
            xf = lnpool.tile([128, NKT, TOWN], BF16, tag="xf", name="xf")
            with tc.tile_pool(name="lnpsf", bufs=1, space="PSUM") as lnps:
                _ln_phase(tc, nc, "lf", xt, lnf_sb[:, 0, :], lnf_sb[:, 1, :],
                          xf, small, scratch, lnps)

            # vocab blocks of 4 share each stationary (xf) load; vector-only
            # eviction keeps Scalar out of the lm pipeline.
            with tc.tile_pool(name="lmw", bufs=2) as lmwp, \
                 tc.tile_pool(name="lmps", bufs=2, space="PSUM") as lmps, \
                 tc.tile_pool(name="lmev", bufs=6) as lmev:
                for vb in range(0, NVC, 4):
                    vcs = list(range(vb, min(vb + 4, NVC)))
                    wts = []
                    for j, vc in enumerate(vcs):
                        wt = lmwp.tile([128, NKT, 512], BF16, tag=f"lmw{j}",
                                       name=f"lmw{j}")
                        nc.sync.dma_start(
                            wt, lmw_d[:, vc * 512:(vc + 1) * 512]
                            .rearrange("(t p) v -> p t v", p=128))
                        wts.append(wt)
                    for tt in range(NOS):
                        pss = [lmps.tile([128, 512], F32, tag=f"lmps{j}",
                                         name=f"lmps{j}")
                               for j in range(len(vcs))]
                        for kt in range(NKT):
                            for j in range(len(vcs)):
                                nc.tensor.matmul(
                                    pss[j], xf[:, kt, tt * 128:(tt + 1) * 128],
                                    wts[j][:, kt, :],
                                    start=(kt == 0), stop=(kt == NKT - 1))
                        for j, vc in enumerate(vcs):
                            ev = lmev.tile([128, 512], F16, tag="lmev",
                                           name="lmev")
                            nc.vector.tensor_copy(ev, pss[j])
                            nc.sync.dma_start(
                                out_d[tt * 128:(tt + 1) * 128,
                                      vc * 512:(vc + 1) * 512], ev)
    nc.finalize()
    return nc


_NC_CACHE = None


def _get_nc():
    global _NC_CACHE
    if _NC_CACHE is None:
        _NC_CACHE = build_bass()
    return _NC_CACHE


def make_in_maps(idx, layer_num, wte, wpe, ln1_g, ln1_b, attn_w, attn_b, proj_w,
                 proj_b, ln2_g, ln2_b, fc_w, fc_b, fc2_w, fc2_b, lnf_g, lnf_b, lm_w):
    bf = ml_dtypes.bfloat16
    idx = np.asarray(idx)
    f32 = np.float32
    wte = np.asarray(wte, f32)
    wpe = np.asarray(wpe, f32)
    x0 = wte[idx] + wpe[:T]                      # [B,T,D] fp32 host embedding

    qkw = np.ascontiguousarray(np.asarray(attn_w, f32)[:, :, :2 * D]).astype(bf)
    vw = np.ascontiguousarray(np.asarray(attn_w, f32)[:, :, 2 * D:]).astype(bf)
    pw = np.asarray(proj_w, f32).astype(bf)
    fcw = np.asarray(fc_w, f32).astype(bf)
    fc2w = np.asarray(fc2_w, f32).astype(bf)
    qkb = np.ascontiguousarray(np.asarray(attn_b, f32)[:, :2 * D])
    vb = np.ascontiguousarray(np.asarray(attn_b, f32)[:, 2 * D:]).astype(bf)
    lnp = np.stack([np.asarray(ln1_g, f32), np.asarray(ln1_b, f32),
                    np.asarray(ln2_g, f32), np.asarray(ln2_b, f32)],
                   axis=1)                        # [L, 4, D] f32
    lnf = np.stack([np.asarray(lnf_g, f32), np.asarray(lnf_b, f32)], axis=0)

    lmw_pad = np.zeros((D, VPAD), f32)
    lmw_pad[:, :V] = np.asarray(lm_w, f32)
    lmw_bf = lmw_pad.astype(bf)

    in_maps = []
    for core in range(8):
        b = core // 2
        p = core % 2
        own = np.concatenate([np.arange(128) + (2 * s + p) * 128
                              for s in range(NOS)])          # own global tokens
        # per-strip tail mask: cols 0:128 = key tile kt=2s, cols 128:256 =
        # kt=2s+1, vs own q chunk 2s+p (pattern is s-independent)
        diag = (np.arange(128)[:, None] <= np.arange(128)[None, :])
        mask = np.empty((128, 256), np.float32)
        if p == 0:
            mask[:, :128] = diag
            mask[:, 128:] = 0.0
        else:
            mask[:, :128] = 1.0
            mask[:, 128:] = diag
        mask = mask.astype(bf)
        in_maps.append(dict(
            xT=np.ascontiguousarray(x0[b].T[:, own]),
            qkw=qkw, vw=vw, pw=pw, fcw=fcw, fc2w=fc2w,
            qkb=qkb, vb=vb, pb=np.asarray(proj_b, f32),
            fcb=np.asarray(fc_b, f32), fc2b=np.asarray(fc2_b, f32),
            lnp=lnp, lnf=lnf, mask=mask,
            lmw=lmw_bf,
        ))
    return in_maps


def kernel(**inputs):
    global LAST_RESULT
    in_maps = make_in_maps(**inputs)
    nc = _get_nc()
    res = run_bass_kernel_spmd(nc, in_maps, core_ids=list(range(8)), trace=TRACE)
    LAST_RESULT = res

    logits = np.empty((B, T, V), np.float32)
    for core in range(8):
        b = core // 2
        p = core % 2
        o = res.results[core]["out"].astype(np.float32)      # [TOWN, VPAD]
        for s in range(NOS):
            g = 2 * s + p
            logits[b, g * 128:(g + 1) * 128, :] = o[s * 128:(s + 1) * 128, :V]
    return logits


# revision 26
# speedup vs baseline: 1.2646x; 1.0331x over previous
"""GPT-2-ish forward (B=4, T=1024, D=768, H=12, L=2, V=50257) on 8 trn2 cores.

Sharding: core pair (2b, 2b+1) sequence-parallel over batch b's tokens:
parity p owns interleaved 128-token chunks {p, p+2, p+4, p+6} (512 tokens).
Per layer each core LNs its own tokens, AllGathers h within the pair (two
256-token chunks, pipelined against QKV compute), computes K/V for all 1024
tokens and Q/attention/proj/MLP for its own 512. lm_head: own tokens x full
vocab (padded to 50688), so no final exchange is needed.

On-device layout: activations [features, tokens]; residual fp32 resident.
Attention: scores per (head, key-tile) with kT stationary; exp on Scalar
(unnormalized, causal mask multiplied after); att@V with V stationary
[128, 65] (ones column appended so the softmax denominator lands in psum
partition 64); normalization = reciprocal + gpsimd partition_broadcast +
vector multiply, emitting attoT directly in [hd, tokens] layout (no
transposes). LayerNorm: column sums via ones-vector matmuls, mean/rstd
broadcasts materialized as rank-1 matmuls in PSUM, apply fused as
sub / mult / two-scalar tensor_scalar on Vector. All matmuls bf16 with
fp32 PSUM; logits evicted f16 and upcast on host.
"""

import numpy as np
import ml_dtypes
from contextlib import ExitStack

import concourse.bass as bass
from concourse import bacc
import concourse.mybir as mybir
import concourse.tile as tile
from concourse.bass_utils import run_bass_kernel_spmd

BF16 = mybir.dt.bfloat16
F32 = mybir.dt.float32
F16 = mybir.dt.float16
AF = mybir.ActivationFunctionType
ALU = mybir.AluOpType

V = 50257
VPAD = 50688          # 99 * 512
D = 768
H = 12
HD = 64
L = 2
T = 1024
B = 4
TOWN = 512            # tokens owned per core
EPS = 1e-5
NKT = D // 128        # 6 feature tiles
NTT = T // 128        # 8 global token tiles
NOS = TOWN // 128     # 4 own token sub-chunks
NVC = VPAD // 512     # 99 lm vocab chunks
PAIRS = [[0, 1], [2, 3], [4, 5], [6, 7]]

TRACE = False
LAST_RESULT = None

_S = {}


def _ln_phase(tc, nc, tag, xt, g_col, b_col, hout, small, scratch, lnps,
              post_half=None):
    """LayerNorm over features (partition dim) of own tokens, processed in
    two 256-token column halves so consumers (the AllGather) start early.
    xt: [128, NKT, TOWN] f32; g_col/b_col: [128, NKT] f32 per-feature params.
    hout: [128, NKT, TOWN] bf16. post_half(ch) is called after half ch."""
    ones_bf = _S["ones_bf"]
    ones_row = _S["ones_row"]
    eps_sb = _S["eps_sb"]

    for ch in range(2):
        cs = slice(ch * 256, (ch + 1) * 256)
        # full-bank padding: s1/s2 accumulation groups interleave, and a
        # start=True clears has_written bank-wide — they must not share one
        s1 = lnps.tile([1, 256], F32, tag="s1", name="s1",
                       padded_shape=[128, 512])
        s2 = lnps.tile([1, 256], F32, tag="s2", name="s2",
                       padded_shape=[128, 512])
        for kt in range(NKT):
            xbf = scratch.tile([128, 256], BF16, tag="xbf", name="xbf")
            sq = scratch.tile([128, 256], BF16, tag="sq", name="sq")
            xs = xt[:, kt, cs]
            nc.vector.tensor_copy(xbf, xs)
            nc.vector.tensor_mul(sq, xs, xs)
            nc.tensor.matmul(s1, ones_bf, xbf, start=(kt == 0),
                             stop=(kt == NKT - 1))
            nc.tensor.matmul(s2, ones_bf, sq, start=(kt == 0),
                             stop=(kt == NKT - 1))
        # mean = s1/D ; var = s2/D - mean^2 ; rstd = 1/sqrt(var+eps)
        mean = small.tile([1, 256], F32, tag="mean", name="mean")
        var = small.tile([1, 256], F32, tag="var", name="var")
        rstd = small.tile([1, 256], F32, tag="rstd", name="rstd")
        mean_bf = small.tile([1, 256], BF16, tag="mean_bf", name="mean_bf")
        rstd_bf = small.tile([1, 256], BF16, tag="rstd_bf", name="rstd_bf")
        nc.vector.tensor_scalar_mul(mean, s1, 1.0 / D)
        nc.vector.tensor_mul(var, mean, mean)
        nc.vector.scalar_tensor_tensor(var, s2, 1.0 / D, var,
                                       op0=ALU.mult, op1=ALU.subtract)
        nc.scalar.activation(var, var, AF.Sqrt, bias=eps_sb)
        nc.vector.reciprocal(rstd, var)
        nc.vector.tensor_copy(mean_bf, mean)
        nc.vector.tensor_copy(rstd_bf, rstd)
        # broadcast fields mb = 1 (x) mean, rb = 1 (x) rstd  [128, 256] psum
        mb = lnps.tile([128, 256], F32, tag="mb", name="mb")
        rb = lnps.tile([128, 256], F32, tag="rb", name="rb")
        nc.tensor.matmul(mb, ones_row[0:1, 0:128], mean_bf, start=True,
                         stop=True)
        nc.tensor.matmul(rb, ones_row[0:1, 0:128], rstd_bf, start=True,
                         stop=True)
        # apply: h = ((x - mb) * rb) * g + b
        for kt in range(NKT):
            tmp = scratch.tile([128, 256], F32, tag="lntmp", name="lntmp")
            nc.vector.tensor_sub(tmp, xt[:, kt, cs], mb)
            nc.vector.tensor_mul(tmp, tmp, rb)
            nc.vector.tensor_scalar(hout[:, kt, cs], tmp,
                                    g_col[:, kt:kt + 1], b_col[:, kt:kt + 1],
                                    op0=ALU.mult, op1=ALU.add)
        if post_half is not None:
            post_half(ch)


def build_bass():
    nc = bacc.Bacc(None, target_bir_lowering=False)
    # ---- DRAM I/O (per-core) ----
    xT_d = nc.dram_tensor("xT", [D, TOWN], F32, kind="ExternalInput")
    h0_d = nc.dram_tensor("h0", [D, T], BF16, kind="ExternalInput")
    h0own_d = nc.dram_tensor("h0own", [D, TOWN], BF16, kind="ExternalInput")
    qkw_d = nc.dram_tensor("qkw", [L, D, 2 * D], BF16, kind="ExternalInput")
    vw_d = nc.dram_tensor("vw", [L, D, D], BF16, kind="ExternalInput")
    pw_d = nc.dram_tensor("pw", [L, D, D], BF16, kind="ExternalInput")
    fcw_d = nc.dram_tensor("fcw", [L, D, 4 * D], BF16, kind="ExternalInput")
    fc2w_d = nc.dram_tensor("fc2w", [L, 4 * D, D], BF16, kind="ExternalInput")
    qkb_d = nc.dram_tensor("qkb", [L, 2 * D], F32, kind="ExternalInput")
    vb_d = nc.dram_tensor("vb", [L, D], BF16, kind="ExternalInput")
    pb_d = nc.dram_tensor("pb", [L, D], F32, kind="ExternalInput")
    fcb_d = nc.dram_tensor("fcb", [L, 4 * D], F32, kind="ExternalInput")
    fc2b_d = nc.dram_tensor("fc2b", [L, D], F32, kind="ExternalInput")
    ln_d = nc.dram_tensor("lnp", [L, 4, D], F32, kind="ExternalInput")  # g1,b1,g2,b2
    lnf_d = nc.dram_tensor("lnf", [2, D], F32, kind="ExternalInput")
    mask_d = nc.dram_tensor("mask", [128, 256], BF16, kind="ExternalInput")
    lmw_d = nc.dram_tensor("lmw", [D, VPAD], BF16, kind="ExternalInput")
    out_d = nc.dram_tensor("out", [TOWN, VPAD], F16, kind="ExternalOutput")

    with tile.TileContext(nc) as tc, ExitStack() as octx:
        singles = octx.enter_context(tc.tile_pool(name="singles", bufs=1))
        resid = octx.enter_context(tc.tile_pool(name="resid", bufs=1))
        dram = octx.enter_context(tc.tile_pool(name="dram", bufs=2, space="DRAM"))

        ones_bf = singles.tile([128, 1], BF16)
        nc.vector.memset(ones_bf, 1.0)
        ones_row = singles.tile([1, 512], BF16)
        nc.vector.memset(ones_row, 1.0)
        eps_sb = singles.tile([1, 1], F32)
        nc.vector.memset(eps_sb, EPS)
        _S["ones_bf"] = ones_bf
        _S["ones_row"] = ones_row
        _S["eps_sb"] = eps_sb

        # mask for the last two key tiles of any strip s: cols 0:128 = tile
        # kt=2s (p=0: lower-tri, p=1: ones), cols 128:256 = tile kt=2s+1
        # (p=0: zeros, p=1: lower-tri)
        mask_sb = singles.tile([128, 256], BF16)
        nc.sync.dma_start(mask_sb, mask_d[:, :])

        # residual stream (own tokens), fp32, resident
        xt = resid.tile([128, NKT, TOWN], F32)
        nc.sync.dma_start(xt, xT_d.rearrange("(k p) t -> p k t", p=128))

        for l in range(L):
            with ExitStack() as lctx:
                lnpool = lctx.enter_context(tc.tile_pool(name=f"ln{l}", bufs=1))
                wpool = lctx.enter_context(tc.tile_pool(name=f"w{l}", bufs=3))
                biasp = lctx.enter_context(tc.tile_pool(name=f"bias{l}", bufs=1))
                small = lctx.enter_context(tc.tile_pool(name=f"small{l}", bufs=2))
                scratch = lctx.enter_context(tc.tile_pool(name=f"scr{l}", bufs=3))

                qkb_sb = biasp.tile([128, 12], F32)
                nc.sync.dma_start(qkb_sb, qkb_d[l].rearrange("(t p) -> p t", p=128))
                vbbf_sb = biasp.tile([1, D], BF16)
                nc.sync.dma_start(vbbf_sb, vb_d[l].rearrange("(o d) -> o d", o=1))
                pb_sb = biasp.tile([128, 6], F32)
                nc.sync.dma_start(pb_sb, pb_d[l].rearrange("(t p) -> p t", p=128))
                fcb_sb = biasp.tile([128, 24], F32)
                nc.sync.dma_start(fcb_sb, fcb_d[l].rearrange("(t p) -> p t", p=128))
                fc2b_sb = biasp.tile([128, 6], F32)
                nc.sync.dma_start(fc2b_sb, fc2b_d[l].rearrange("(t p) -> p t", p=128))
                ln_sb = biasp.tile([128, 4, NKT], F32)
                nc.sync.dma_start(ln_sb, ln_d[l].rearrange("g (k p) -> p g k", p=128))

                # ---------- LN1 (own tokens) + AllGather h within pair ----
                # hfull split per gathered chunk so chunk-0 consumers do not
                # wait on the chunk-1 collective (dep tracking is per-tile).
                h = lnpool.tile([128, NKT, TOWN], BF16, tag="h", name="h")
                hfull = [lnpool.tile([128, NKT, 512], BF16, tag=f"hfull{c}",
                                     name=f"hfull{c}") for c in range(2)]

                def kick_ag(ch):
                    cs = slice(ch * 256, (ch + 1) * 256)
                    agin = dram.tile([D, 256], BF16, tag="agin", name="agin")
                    agout = dram.tile([2, D, 256], BF16, tag="agout", name="agout")
                    nc.sync.dma_start(
                        agin.rearrange("(k p) t -> p k t", p=128), h[:, :, cs])
                    nc.gpsimd.collective_compute(
                        "AllGather", ALU.bypass, replica_groups=PAIRS,
                        ins=[agin.opt()], outs=[agout.opt()])
                    # global chunk 4ch + 2s + r lands at block position s*256+r*128
                    hdst = hfull[ch].rearrange("p k (s r t) -> r p k s t",
                                               s=2, r=2, t=128)
                    for r in range(2):
                        asrc = agout[r].rearrange("(k p) (s t) -> s p k t",
                                                  p=128, t=128)
                        for s in range(2):
                            nc.sync.dma_start(hdst[r, :, :, s, :], asrc[s])

                if l == 0:
                    # layer 0: LN1(x0) is precomputed on host — no collective
                    nc.sync.dma_start(
                        h, h0own_d.rearrange("(k p) t -> p k t", p=128))
                    for c in range(2):
                        nc.sync.dma_start(
                            hfull[c], h0_d[:, c * 512:(c + 1) * 512]
                            .rearrange("(k p) t -> p k t", p=128))
                else:
                    with tc.tile_pool(name=f"lnps{l}a", bufs=1,
                                      space="PSUM") as lnps:
                        _ln_phase(tc, nc, f"l{l}a", xt, ln_sb[:, 0, :],
                                  ln_sb[:, 1, :], h, small, scratch, lnps,
                                  post_half=kick_ag)

                # ---------- qT own, kT/V full, early scores --------------
                # Attention strips: per (pr, hh, s) the scores for own
                # q-sub-chunk s (128 cols) cover key tiles kt=0..2s+1 and are
                # packed [128, (2s+2)*128] in psum; exp evicts to bf16 strips.
                # s=0,1 (kt<=3, needs only AG chunk 0) run for all pr between
                # kT chunk 0 and kT chunk 1 to cover the AllGather latency.
                q_sb = lnpool.tile([128, NKT, TOWN], BF16, tag="q_sb", name="q_sb")
                k_sb = lnpool.tile([128, NKT, T], BF16, tag="k_sb", name="k_sb")
                attT01 = lnpool.tile([128, NKT, 2, 768], BF16, tag="attT01",
                                     name="attT01")
                v_aug = [lnpool.tile([128, 12, 65], BF16, tag=f"vaug{i}",
                                     name=f"vaug{i}") for i in range(NTT)]
                fcw_sb = wpool.tile([128, NKT, 4 * D], BF16, tag="fcw_l",
                                    name="fcw_sb", bufs=1)

                def scores_strip(sps_pool, stag, twid, pr, hh, s, dst):
                    """Scores+exp+mask for strip (pr, hh, s) -> dst bf16 AP."""
                    nk = 2 * s + 2
                    hs = slice(hh * 64, hh * 64 + 64)
                    st = sps_pool.tile([128, twid], F32, tag=stag, name=stag)
                    for kt in range(nk):
                        nc.tensor.matmul(
                            st[:, kt * 128:(kt + 1) * 128],
                            k_sb[hs, pr, kt * 128:(kt + 1) * 128],
                            q_sb[hs, pr, s * 128:(s + 1) * 128],
                            start=True, stop=True)
                    for a0 in range(0, nk * 128, 512):
                        a1 = min(nk * 128, a0 + 512)
                        nc.scalar.activation(dst[:, a0:a1], st[:, a0:a1],
                                             AF.Exp, scale=0.125)
                    # only the last two key tiles (kt=2s, 2s+1) need masking
                    nc.vector.tensor_mul(dst[:, (nk - 2) * 128:nk * 128],
                                         dst[:, (nk - 2) * 128:nk * 128],
                                         mask_sb)

                def kv_chunk(qkps, ch):
                    gs = slice(ch * 512, (ch + 1) * 512)
                    for f in range(NKT):
                        wt = wpool.tile([128, NKT, 128], BF16, tag="kw_t",
                                        name="kw_t", bufs=2)
                        nc.sync.dma_start(
                            wt, qkw_d[l][:, D + f * 128:D + (f + 1) * 128]
                            .rearrange("(t p) f -> p t f", p=128))
                        ps = qkps.tile([128, 512], F32, tag="qkps", name="qkps")
                        for kt in range(NKT):
                            nc.tensor.matmul(ps, wt[:, kt, :], hfull[ch][:, kt, :],
                                             start=(kt == 0),
                                             stop=(kt == NKT - 1))
                        nc.vector.tensor_scalar_add(k_sb[:, f, gs], ps,
                                                    qkb_sb[:, 6 + f:7 + f])
                    for tt in range(ch * 4, ch * 4 + 4):
                        nc.vector.memset(v_aug[tt][:, :, 64:65], 1.0)
                        for vc in range(2):
                            vs = slice(vc * 384, (vc + 1) * 384)
                            ps = qkps.tile([128, 384], F32, tag="vps", name="vps",
                                           bufs=2)
                            for kt in range(NKT):
                                nc.tensor.matmul(
                                    ps, hfull[ch][:, kt,
                                                  (tt % 4) * 128:(tt % 4 + 1) * 128],
                                    vw_sb[kt][:, vs],
                                    start=(kt == 0), stop=False)
                            nc.tensor.matmul(ps, ones_row[:, 0:128],
                                             vbbf_sb[:, vs],
                                             start=False, stop=True)
                            nc.vector.tensor_copy(
                                v_aug[tt][:, vc * 6:(vc + 1) * 6, 0:64],
                                ps.rearrange("p (h d) -> p h d", d=64))

                with tc.tile_pool(name=f"qkps{l}", bufs=3, space="PSUM") as qkps:
                    for f in range(NKT):
                        wt = wpool.tile([128, NKT, 128], BF16, tag="qw_t",
                                        name="qw_t", bufs=2)
                        nc.sync.dma_start(
                            wt, qkw_d[l][:, f * 128:(f + 1) * 128]
                            .rearrange("(t p) f -> p t f", p=128))
                        ps = qkps.tile([128, TOWN], F32, tag="qkps", name="qkps")
                        for kt in range(NKT):
                            nc.tensor.matmul(ps, wt[:, kt, :], h[:, kt, :],
                                             start=(kt == 0), stop=(kt == NKT - 1))
                        nc.vector.tensor_scalar_add(q_sb[:, f, :], ps,
                                                    qkb_sb[:, f:f + 1])

                    vw_sb = [wpool.tile([128, D], BF16, tag=f"vw{i}",
                                        name=f"vw{i}", bufs=1) for i in range(NKT)]
                    for kt in range(NKT):
                        nc.sync.dma_start(vw_sb[kt],
                                          vw_d[l][kt * 128:(kt + 1) * 128, :])
                    kv_chunk(qkps, 0)
                    # early scores s=0,1 for all pr (covers AG chunk 1 wait)
                    nc.sync.dma_start(fcw_sb, fcw_d[l]
                                      .rearrange("(t p) f -> p t f", p=128))
                    for pr in range(NKT):
                        for hh in range(2):
                            for s in range(2):
                                scores_strip(
                                    qkps, "stripA", TOWN, pr, hh, s,
                                    attT01[:, pr, hh,
                                           s * 256:s * 256 + (2 * s + 2) * 128])
                    kv_chunk(qkps, 1)

                # ---------- attention tail per head-pair ------------------
                attoT = lnpool.tile([128, NKT, TOWN], BF16, tag="attoT",
                                    name="attoT")
                with tc.tile_pool(name=f"sps{l}", bufs=2, space="PSUM") as sps, \
                     tc.tile_pool(name=f"ops{l}", bufs=2, space="PSUM") as ops, \
                     tc.tile_pool(name=f"attp{l}", bufs=3) as attp:
                    attT23s = {}

                    def attv(pr, hh):
                        hcol = 2 * pr + hh
                        attT23 = attT23s[pr]

                        def att_src(s, kt):
                            if s < 2:
                                base = s * 256
                                return attT01[:, pr, hh,
                                              base + kt * 128:
                                              base + (kt + 1) * 128]
                            base = (s - 2) * 768
                            return attT23[:, hh, base + kt * 128:
                                          base + (kt + 1) * 128]

                        po = ops.tile([65, TOWN], F32, tag=f"po{hh}",
                                      name=f"po{hh}")
                        # start=True clears has_written for the whole bank,
                        # so only the first MM opens the group; later
                        # regions overwrite-on-first-touch.
                        for kt in range(NTT):
                            for s in range(kt // 2, 4):
                                nc.tensor.matmul(
                                    po[:, s * 128:(s + 1) * 128],
                                    v_aug[kt][:, hcol, :], att_src(s, kt),
                                    start=(kt == 0 and s == 0),
                                    stop=(kt == 2 * s + 1),
                                    skip_group_check=True)
                        r_sb = scratch.tile([1, TOWN], F32, tag="r_sb",
                                            name="r_sb")
                        rbc = scratch.tile([64, TOWN], F32, tag="rbc",
                                           name="rbc")
                        nc.vector.reciprocal(r_sb, po[64:65, :])
                        nc.gpsimd.partition_broadcast(rbc, r_sb, channels=64)
                        nc.vector.tensor_mul(
                            attoT[hh * 64:hh * 64 + 64, pr, :],
                            po[0:64, :], rbc)

                    # software pipeline: chain i's scores overlap chain i-1's
                    # att@V, keeping the PE dense while exp chases.
                    chains = [(pr, hh) for pr in range(NKT) for hh in range(2)]
                    for i, (pr, hh) in enumerate(chains):
                        if hh == 0:
                            attT23s[pr] = attp.tile([128, 2, 1792], BF16,
                                                    tag="attT23", name="attT23")
                        for s in range(2, 4):
                            scores_strip(
                                sps, "stripB", 1024, pr, hh, s,
                                attT23s[pr][:, hh, (s - 2) * 768:
                                            (s - 2) * 768 + (2 * s + 2) * 128])
                        if i >= 1:
                            attv(*chains[i - 1])
                    attv(*chains[-1])

                # ---------- proj + residual ----------
                with tc.tile_pool(name=f"pps{l}", bufs=3, space="PSUM") as pps:
                    for ot in range(NKT):
                        wt = wpool.tile([128, NKT, 128], BF16, tag="pw_t",
                                        name="pw_t")
                        nc.sync.dma_start(
                            wt, pw_d[l][:, ot * 128:(ot + 1) * 128]
                            .rearrange("(t p) f -> p t f", p=128))
                        ps = pps.tile([128, TOWN], F32, tag="pps", name="pps")
                        for kt in range(NKT):
                            nc.tensor.matmul(ps, wt[:, kt, :], attoT[:, kt, :],
                                             start=(kt == 0), stop=(kt == NKT - 1))
                        nc.vector.scalar_tensor_tensor(
                            xt[:, ot, :], ps, pb_sb[:, ot:ot + 1],
                            xt[:, ot, :], op0=ALU.add, op1=ALU.add)

                # ---------- LN2 + MLP (own tokens) ----------
                h2 = lnpool.tile([128, NKT, TOWN], BF16, tag="h", name="h2")
                with tc.tile_pool(name=f"lnps{l}b", bufs=1, space="PSUM") as lnps:
                    _ln_phase(tc, nc, f"l{l}b", xt, ln_sb[:, 2, :], ln_sb[:, 3, :],
                              h2, small, scratch, lnps)

                with tc.tile_pool(name=f"mlpps{l}", bufs=3, space="PSUM") as mlpps, \
                     tc.tile_pool(name=f"h2p{l}", bufs=1) as h2p:
                    h2c = h2p.tile([128, 24, TOWN], BF16, tag="h2c", name="h2c")
                    for f in range(24):
                        ps = mlpps.tile([128, TOWN], F32, tag="fcps", name="fcps")
                        for kt in range(NKT):
                            nc.tensor.matmul(ps, fcw_sb[:, kt, f * 128:(f + 1) * 128],
                                             h2[:, kt, :],
                                             start=(kt == 0), stop=(kt == NKT - 1))
                        nc.scalar.activation(h2c[:, f, :], ps, AF.Gelu_apprx_tanh,
                                             bias=fcb_sb[:, f:f + 1])
                    for ot in range(NKT):
                        wt = wpool.tile([128, 24, 128], BF16, tag="fc2w_t",
                                        name="fc2w_t", bufs=2)
                        nc.sync.dma_start(
                            wt, fc2w_d[l][:, ot * 128:(ot + 1) * 128]
                            .rearrange("(t p) f -> p t f", p=128))
                        ps = mlpps.tile([128, TOWN], F32, tag="fc2ps", name="fc2ps")
                        for kt in range(24):
                            nc.tensor.matmul(ps, wt[:, kt, :], h2c[:, kt, :],
                                             start=(kt == 0), stop=(kt == 23))
                        nc.vector.scalar_tensor_tensor(
                            xt[:, ot, :], ps, fc2b_sb[:, ot:ot + 1],
                            xt[:, ot, :], op0=ALU.add, op1=ALU.add)

        # ---------- final LN + lm_head (own tokens, full vocab) ----------
        with ExitStack() as fctx:
            lnpool = fctx.enter_context(tc.tile_pool(name="lnfp", bufs=1))
            biasp = fctx.enter_context(tc.tile_pool(name="biasf", bufs=1))
            small = fctx.enter_context(tc.tile_pool(name="smallf", bufs=2))
            scratch = fctx.enter_context(tc.tile_pool(name="scrf", bufs=3))
            lnf_sb = biasp.tile([128, 2, NKT], F32)
            nc.sync.dma_start(lnf_sb, lnf_d.rearrange("g (k p) -> p g k", p=128))
            xf = lnpool.tile([128, NKT, TOWN], BF16, tag="xf", name="xf")
            with tc.tile_pool(name="lnpsf", bufs=1, space="PSUM") as lnps:
                _ln_phase(tc, nc, "lf", xt, lnf_sb[:, 0, :], lnf_sb[:, 1, :],
                          xf, small, scratch, lnps)

            # vocab blocks of 4 share each stationary (xf) load; vector-only
            # eviction keeps Scalar out of the lm pipeline.
            with tc.tile_pool(name="lmw", bufs=2) as lmwp, \
                 tc.tile_pool(name="lmps", bufs=2, space="PSUM") as lmps, \
                 tc.tile_pool(name="lmev", bufs=6) as lmev:
                for vb in range(0, NVC, 4):
                    vcs = list(range(vb, min(vb + 4, NVC)))
                    wts = []
                    for j, vc in enumerate(vcs):
                        wt = lmwp.tile([128, NKT, 512], BF16, tag=f"lmw{j}",
                                       name=f"lmw{j}")
                        nc.sync.dma_start(
                            wt, lmw_d[:, vc * 512:(vc + 1) * 512]
                            .rearrange("(t p) v -> p t v", p=128))
                        wts.append(wt)
                    for tt in range(NOS):
                        pss = [lmps.tile([128, 512], F32, tag=f"lmps{j}",
                                         name=f"lmps{j}")
                               for j in range(len(vcs))]
                        for kt in range(NKT):
                            for j in range(len(vcs)):
                                nc.tensor.matmul(
                                    pss[j], xf[:, kt, tt * 128:(tt + 1) * 128],
                                    wts[j][:, kt, :],
                                    start=(kt == 0), stop=(kt == NKT - 1))
                        for j, vc in enumerate(vcs):
                            ev = lmev.tile([128, 512], F16, tag="lmev",
                                           name="lmev")
                            nc.vector.tensor_copy(ev, pss[j])
                            nc.sync.dma_start(
                                out_d[tt * 128:(tt + 1) * 128,
                                      vc * 512:(vc + 1) * 512], ev)
    nc.finalize()
    return nc


_NC_CACHE = None


def _get_nc():
    global _NC_CACHE
    if _NC_CACHE is None:
        _NC_CACHE = build_bass()
    return _NC_CACHE


def make_in_maps(idx, layer_num, wte, wpe, ln1_g, ln1_b, attn_w, attn_b, proj_w,
                 proj_b, ln2_g, ln2_b, fc_w, fc_b, fc2_w, fc2_b, lnf_g, lnf_b, lm_w):
    bf = ml_dtypes.bfloat16
    idx = np.asarray(idx)
    f32 = np.float32
    wte = np.asarray(wte, f32)
    wpe = np.asarray(wpe, f32)
    x0 = wte[idx] + wpe[:T]                      # [B,T,D] fp32 host embedding
    # layer-0 LN1 on host (saves the first AllGather on device)
    mu = x0.mean(-1, keepdims=True)
    var = x0.var(-1, keepdims=True)
    h0 = ((x0 - mu) / np.sqrt(var + EPS) * np.asarray(ln1_g, f32)[0]
          + np.asarray(ln1_b, f32)[0]).astype(bf)           # [B,T,D]

    qkw = np.ascontiguousarray(np.asarray(attn_w, f32)[:, :, :2 * D]).astype(bf)
    vw = np.ascontiguousarray(np.asarray(attn_w, f32)[:, :, 2 * D:]).astype(bf)
    pw = np.asarray(proj_w, f32).astype(bf)
    fcw = np.asarray(fc_w, f32).astype(bf)
    fc2w = np.asarray(fc2_w, f32).astype(bf)
    qkb = np.ascontiguousarray(np.asarray(attn_b, f32)[:, :2 * D])
    vb = np.ascontiguousarray(np.asarray(attn_b, f32)[:, 2 * D:]).astype(bf)
    lnp = np.stack([np.asarray(ln1_g, f32), np.asarray(ln1_b, f32),
                    np.asarray(ln2_g, f32), np.asarray(ln2_b, f32)],
                   axis=1)                        # [L, 4, D] f32
    lnf = np.stack([np.asarray(lnf_g, f32), np.asarray(lnf_b, f32)], axis=0)

    lmw_pad = np.zeros((D, VPAD), f32)
    lmw_pad[:, :V] = np.asarray(lm_w, f32)
    lmw_bf = lmw_pad.astype(bf)

    in_maps = []
    for core in range(8):
        b = core // 2
        p = core % 2
        own = np.concatenate([np.arange(128) + (2 * s + p) * 128
                              for s in range(NOS)])          # own global tokens
        # per-strip tail mask: cols 0:128 = key tile kt=2s, cols 128:256 =
        # kt=2s+1, vs own q chunk 2s+p (pattern is s-independent)
        diag = (np.arange(128)[:, None] <= np.arange(128)[None, :])
        mask = np.empty((128, 256), np.float32)
        if p == 0:
            mask[:, :128] = diag
            mask[:, 128:] = 0.0
        else:
            mask[:, :128] = 1.0
            mask[:, 128:] = diag
        mask = mask.astype(bf)
        in_maps.append(dict(
            xT=np.ascontiguousarray(x0[b].T[:, own]),
            h0=np.ascontiguousarray(h0[b].T),
            h0own=np.ascontiguousarray(h0[b].T[:, own]),
            qkw=qkw, vw=vw, pw=pw, fcw=fcw, fc2w=fc2w,
            qkb=qkb, vb=vb, pb=np.asarray(proj_b, f32),
            fcb=np.asarray(fc_b, f32), fc2b=np.asarray(fc2_b, f32),
            lnp=lnp, lnf=lnf, mask=mask,
            lmw=lmw_bf,
        ))
    return in_maps


def kernel(**inputs):
    global LAST_RESULT
    in_maps = make_in_maps(**inputs)
    nc = _get_nc()
    res = run_bass_kernel_spmd(nc, in_maps, core_ids=list(range(8)), trace=TRACE)
    LAST_RESULT = res

    logits = np.empty((B, T, V), np.float32)
    for core in range(8):
        b = core // 2
        p = core % 2
        o = res.results[core]["out"].astype(np.float32)      # [TOWN, VPAD]
        for s in range(NOS):
            g = 2 * s + p
            logits[b, g * 128:(g + 1) * 128, :] = o[s * 128:(s + 1) * 128, :V]
    return logits
